# revision 5
# baseline (speedup 1.0000x reference)
"""Causal multi-head flash-attention block (QKV proj + attention + out proj)
for Trainium2, distributed over 8 NeuronCores.

Sharding: data-parallel over batch (B=4) x tensor-parallel over head groups
(16 heads -> 2 groups of 8). Core c handles batch c//2, head group c%2.
Each core computes a partial output projection (its 8 heads' contribution);
the host sums the two partials per batch and adds the bias.

v2 schedule notes (vs v1): the attention inner loop is ACT(exp)-bound
(~2.2us of scalar-engine work per k-group vs ~1.3us of PE work), and the
PE executes its queue in order, so independent matmul work must be
interleaved INTO the attention group loop to keep the PE dense:
  - pair p+1's QT/KT projection units fill pair p's exp-gated PE gaps,
  - V strips fill pair 0, out-proj chunks fill pair 3,
  - the input-DMA ramp runs pair-0 QT/KT chunk-gated (tile-minor over
    6 live psum accumulators) so the PE starts at ~3us, not ~38us.
Other changes: exp skips dead diagonal columns on the last group of each
q-tile; softmax denominators are extracted with tiny DMAs (not 2us gpsimd
copies); the normalize multiplies run on gpsimd so the DVE FIFO never
head-of-line-blocks PE-feeding casts/masks; partial outputs are written
bf16 (host sums in f32).

Per-core kernel (all matmuls bf16 operands, fp32 PSUM accumulate):
  - QKV proj from host-pretransposed x^T: Q^T,K^T in [d, s] layout, V in
    [s, d] layout with a ones-column per head (rowsum trick).
  - Scores computed transposed: ST[k,q] via lhsT=KT-block, rhs=QT; two
    heads packed in the PE array via row tiling (contraction K=64 each,
    partitions 0:64 / 64:128 share one XBUS).
  - softmax without max-subtraction (logits ~ N(0,1)); exp on ACT with the
    1/8 scale folded in; causal masking by 0/1 mask multiply post-exp on
    the diagonal blocks of each q-tile; fully-masked blocks skipped.
  - AV: lhsT = V-tile [128, 65] (65th col = ones -> row 64 of PSUM
    accumulates the softmax denominator), rhs = P^T tiles.
  - Normalize: row 64 -> partition 0 via tiny DMA, merged reciprocal on
    DVE, partition_broadcast + multiply on gpsimd.
  - Output proj from O^T [head*64+d, s] chunks against w_proj rows.
"""

import numpy as np
import ml_dtypes

import concourse.bass as bass
import concourse.bacc as bacc
import concourse.mybir as mybir
import concourse.tile as tile
from concourse.bass_utils import run_bass_kernel_spmd

F32 = mybir.dt.float32
BF16 = mybir.dt.bfloat16
EXP = mybir.ActivationFunctionType.Exp

# Problem constants (hardcoded per contract)
B, S, C = 4, 2048, 1024
NH, D = 16, 64
SCALE = D ** -0.5
N_CORES = 8
HG = NH // 2          # heads per core (head group)
NPAIR = HG // 2       # head pairs per core
CCH = C // 128        # contraction chunks for QKV proj
SC = S // 128         # s-chunks (also k-blocks count)
NQT = S // 512        # q-tiles of 512
GW = C // 2           # group width of qkv output (8 heads * 64)


def build_nc():
    nc = bacc.Bacc("TRN2", target_bir_lowering=False, debug=False)

    xT = nc.dram_tensor("xT", [C, S], BF16, kind="ExternalInput")
    wq = nc.dram_tensor("wq", [C, GW], BF16, kind="ExternalInput")
    wk = nc.dram_tensor("wk", [C, GW], BF16, kind="ExternalInput")
    wv = nc.dram_tensor("wv", [C, GW], BF16, kind="ExternalInput")
    wp = nc.dram_tensor("wp", [GW, C], BF16, kind="ExternalInput")
    mask = nc.dram_tensor("mask", [128, 512], BF16, kind="ExternalInput")
    out = nc.dram_tensor("out", [S, C], BF16, kind="ExternalOutput")

    with tc_scope(nc) as (tc, cpool, ptpool, wpool, pspool):
        # ---- constant/persistent tiles + input DMAs ----
        # DMA issue order is the arrival order: wq/wk first (QT/KT ramp),
        # then xT in column-halves, then wv, wp, mask.
        wq_sb = [cpool.tile([128, GW], BF16, tag=f"wq{cc}", name=f"wq{cc}")
                 for cc in range(CCH)]
        wk_sb = [cpool.tile([128, GW], BF16, tag=f"wk{cc}", name=f"wk{cc}")
                 for cc in range(CCH)]
        wv_sb = [cpool.tile([128, GW], BF16, tag=f"wv{cc}", name=f"wv{cc}")
                 for cc in range(CCH)]
        xt_sb = [cpool.tile([128, S], BF16, tag=f"xt{cc}", name=f"xt{cc}")
                 for cc in range(CCH)]
        for cc in range(CCH):
            nc.sync.dma_start(wq_sb[cc][:], wq[128 * cc:128 * (cc + 1), :])
            nc.sync.dma_start(wk_sb[cc][:], wk[128 * cc:128 * (cc + 1), :])
        mask_sb = cpool.tile([128, 512], BF16, tag="mask", name="maskt")
        nc.sync.dma_start(mask_sb[:], mask[:, :])
        for cc in range(CCH):
            nc.sync.dma_start(xt_sb[cc][:, 0:1024],
                              xT[128 * cc:128 * (cc + 1), 0:1024])
        for cc in range(CCH):
            nc.sync.dma_start(xt_sb[cc][:, 1024:2048],
                              xT[128 * cc:128 * (cc + 1), 1024:2048])
        for cc in range(CCH):
            nc.sync.dma_start(wv_sb[cc][:], wv[128 * cc:128 * (cc + 1), :])
        wp_sb = []
        for p in range(NPAIR):
            t = cpool.tile([128, C], BF16, tag=f"wp{p}", name=f"wp{p}")
            nc.sync.dma_start(t[:], wp[128 * p:128 * (p + 1), :])
            wp_sb.append(t)
        # preload the ACT exp table set while input DMAs run
        actwarm = cpool.tile([1, 8], F32, tag="actwarm", name="actwarm")
        nc.vector.memset(actwarm[:], 0.0)
        nc.scalar.activation(actwarm[:], actwarm[:], EXP)

        # persistent per-pair QT/KT [d(2 heads stacked), s] bf16
        qt_sb = [cpool.tile([128, S], BF16, tag=f"qt{p}", name=f"qt{p}")
                 for p in range(NPAIR)]
        kt_sb = [cpool.tile([128, S], BF16, tag=f"kt{p}", name=f"kt{p}")
                 for p in range(NPAIR)]

        # O^T normalized, per head pair: head0 partitions 0:64,
        # head1 partitions 64:128 (layout = rows of w_proj)
        otn_sb = [cpool.tile([128, S], BF16, tag=f"otn{p}", name=f"otn{p}")
                  for p in range(NPAIR)]

        vt_sb = [cpool.tile([128, 65 * HG], BF16, tag=f"vt{sc}",
                            name=f"vt{sc}")
                 for sc in range(SC)]

        # ---- QT/KT projection unit: one (pair, st, q|k) output tile ----
        def emit_qkt_unit(p, st, which, ptag):
            ssl = slice(512 * st, 512 * (st + 1))
            w_sb, dst = (wq_sb, qt_sb[p]) if which == 0 else (wk_sb, kt_sb[p])
            ps = pspool.tile([128, 512], F32, tag=ptag, name="qkps", bufs=2)
            for cc in range(CCH):
                nc.tensor.matmul(
                    ps[:], w_sb[cc][:, 128 * p:128 * (p + 1)],
                    xt_sb[cc][:, ssl],
                    start=(cc == 0), stop=(cc == CCH - 1))
            nc.vector.tensor_copy(dst[:, ssl], ps[:])

        # ---- pair-0 ramp: tile-minor over 6 live accumulators so the ----
        # matmuls chase the chunk DMAs instead of waiting for all of xT.
        waveA = [(0, 0, "sps"), (0, 1, "sps"), (1, 0, "ot"), (1, 1, "ot")]
        waveB = [(2, 0, "mm"), (2, 1, "mm"), (3, 0, "sps"), (3, 1, "sps")]
        for wave in (waveA, waveB):
            pss = []
            for st, which, ptag in wave:
                ps = pspool.tile([128, 512], F32, tag=ptag, name="rampps",
                                 bufs=2)
                pss.append(ps)
            for cc in range(CCH):
                for (st, which, ptag), ps in zip(wave, pss):
                    w_sb = wq_sb if which == 0 else wk_sb
                    nc.tensor.matmul(
                        ps[:], w_sb[cc][:, 0:128],
                        xt_sb[cc][:, 512 * st:512 * (st + 1)],
                        start=(cc == 0), stop=(cc == CCH - 1))
            for (st, which, ptag), ps in zip(wave, pss):
                dst = qt_sb[0] if which == 0 else kt_sb[0]
                nc.vector.tensor_copy(dst[:, 512 * st:512 * (st + 1)], ps[:])

        # ---- V strip: V = x @ wv in [s, d] layout + ones column ----
        def emit_v_strip(sc):
            vt = vt_sb[sc]
            nc.gpsimd.memset(vt[:], 1.0)
            ps = pspool.tile([128, GW], F32, tag="mm", name="vps", bufs=2)
            for cc in range(CCH):
                nc.tensor.matmul(
                    ps[:], xt_sb[cc][:, 128 * sc:128 * (sc + 1)],
                    wv_sb[cc][:], start=(cc == 0), stop=(cc == CCH - 1))
            vt_v = vt[:, :].rearrange("p (h d) -> p h d", h=HG)[:, :, 0:64]
            ps_v = ps[:, :].rearrange("p (h d) -> p h d", h=HG)
            nc.vector.tensor_copy(vt_v, ps_v)

        for sc in range(4):
            emit_v_strip(sc)

        # ---- out-proj chunk: out[s-chunk, :] = sum_p OTn_p.T @ wp_p ----
        def emit_out_chunk(sc):
            outst = wpool.tile([128, C], BF16, tag="outst", name="outst")
            for half in range(2):
                pp = pspool.tile([128, 512], F32, tag="mm", name="pp",
                                 bufs=2)
                for p in range(NPAIR):
                    nc.tensor.matmul(
                        pp[:], otn_sb[p][:, 128 * sc:128 * (sc + 1)],
                        wp_sb[p][:, 512 * half:512 * (half + 1)],
                        start=(p == 0), stop=(p == NPAIR - 1))
                nc.vector.tensor_copy(
                    outst[:, 512 * half:512 * (half + 1)], pp[:])
            nc.sync.dma_start(out[128 * sc:128 * (sc + 1), :], outst[:])

        # ---- per head-pair attention ----
        for p in range(NPAIR):
            qt, kt = qt_sb[p], kt_sb[p]
            for j in range(NQT):
                nkb = 4 * (j + 1)  # causal: only k-blocks 0..nkb-1
                ngroups = nkb // 2
                # filler PE work for this (p, j): independent matmul
                # closures interleaved between score groups
                fillers = []
                if p == 0 and j < 3:
                    for sc in range(4 * (j + 1), 4 * (j + 2)):
                        fillers.append(lambda sc=sc: emit_v_strip(sc))
                if p < 3:
                    # next pair's QT/KT: 2 units per j
                    for u in range(2):
                        st, which = divmod(2 * j + u, 2)
                        fillers.append(
                            lambda st=st, w=which, pp_=p + 1:
                            emit_qkt_unit(pp_, st, w, "mm"))
                # spread fillers across the group loop
                fill_at = {}
                for fi in range(len(fillers)):
                    g = min(ngroups - 1, (fi * ngroups) // max(1, len(fillers)))
                    fill_at.setdefault(g, []).append(fillers[fi])
                if p == 3 and j >= 1:
                    # out-proj chunks of q-tile j-1: place in the LAST two
                    # groups -- their otn inputs come from normalize(j-1),
                    # which completes a few us into this j (in-order PE
                    # would stall on them if placed early)
                    for i, sc in enumerate(range(4 * (j - 1), 4 * j)):
                        g = ngroups - 2 + (i % 2)
                        fill_at.setdefault(g, []).append(
                            lambda sc=sc: emit_out_chunk(sc))

                ot = [pspool.tile([65, 512], F32, tag="ot", name="ot",
                                  bufs=2) for _ in range(2)]

                def emit_av(g, pt, j=j, nkb=nkb, ot=ot, p=p):
                    # AV accumulation (65th row = softmax denominator)
                    for kb in (2 * g, 2 * g + 1):
                        o = 128 * (kb - 4 * j) if kb >= 4 * j else 0
                        for h in range(2):
                            nc.tensor.matmul(
                                ot[h][:, o:512],
                                vt_sb[kb][:, 65 * (2 * p + h):
                                          65 * (2 * p + h) + 65],
                                pt[h][:, 512 * (kb % 2) + o:
                                      512 * (kb % 2 + 1)],
                                start=(kb == 0), stop=(kb == nkb - 1))

                pending = []
                for g in range(ngroups):
                    sp = [pspool.tile([128, 1024], F32, tag="sps",
                                      name="sps", bufs=2) for _ in range(2)]
                    # scores (transposed): 2 k-blocks x 2 packed heads.
                    # Diagonal blocks restrict to the causally live
                    # columns [o:512]; dead psum columns hold stale but
                    # bounded old scores and are never consumed.
                    for kb in (2 * g, 2 * g + 1):
                        o = 128 * (kb - 4 * j) if kb >= 4 * j else 0
                        for h in range(2):
                            hsl = slice(64 * h, 64 * (h + 1))
                            nc.tensor.matmul(
                                sp[h][:, 512 * (kb % 2) + o:
                                      512 * (kb % 2 + 1)],
                                kt[hsl, 128 * kb:128 * (kb + 1)],
                                qt[hsl, 512 * j + o:512 * (j + 1)],
                                start=True, stop=True)
                    pt = [ptpool.tile([128, 1024], BF16, tag="pt",
                                      name="pt") for _ in range(2)]
                    if g == ngroups - 1:
                        # last group = k-blocks 4j+2, 4j+3: only columns
                        # [256:512] and [896:1024] are causally live --
                        # exp just those slices (saves ACT time)
                        for h in range(2):
                            nc.scalar.activation(pt[h][:, 256:512],
                                                 sp[h][:, 256:512], EXP,
                                                 scale=SCALE)
                            nc.scalar.activation(pt[h][:, 896:1024],
                                                 sp[h][:, 896:1024], EXP,
                                                 scale=SCALE)
                    else:
                        for h in range(2):
                            nc.scalar.activation(pt[h][:], sp[h][:], EXP,
                                                 scale=SCALE)
                    # causal mask on diagonal blocks (multiplicative)
                    for kb in (2 * g, 2 * g + 1):
                        if kb >= 4 * j:
                            o = 128 * (kb - 4 * j)
                            csl = slice(512 * (kb % 2) + o,
                                        512 * (kb % 2 + 1))
                            for h in range(2):
                                nc.vector.tensor_mul(
                                    pt[h][:, csl], pt[h][:, csl],
                                    mask_sb[:, 0:512 - o])
                    pending.append((g, pt))
                    if len(pending) > 2:
                        emit_av(*pending.pop(0))
                    for f in fill_at.get(g, []):
                        f()
                for item in pending:
                    emit_av(*item)

                # ---- normalize: numerators to SBUF (releases psum),
                # denominators (psum row 64) to partition 0 via tiny DMAs,
                # merged reciprocal on DVE, broadcast+multiply on gpsimd
                # (keeps the DVE FIFO free for PE-feeding casts/masks).
                qsl = slice(512 * j, 512 * (j + 1))
                s64a = wpool.tile([65, 512], F32, tag="s64a", name="s64a")
                s64b = wpool.tile([65, 512], F32, tag="s64b", name="s64b")
                nc.vector.tensor_copy(s64a[:, :], ot[0][:, :])
                nc.vector.tensor_copy(s64b[:, :], ot[1][:, :])
                dens = wpool.tile([1, 1024], F32, tag="dens", name="dens")
                nc.sync.dma_start(dens[0:1, 0:512], s64a[64:65, :])
                nc.sync.dma_start(dens[0:1, 512:1024], s64b[64:65, :])
                inv = wpool.tile([1, 1024], F32, tag="inv", name="inv")
                nc.vector.reciprocal_approx_fast(inv[0:1, :], dens[0:1, :])
                bcs0 = wpool.tile([64, 512], F32, tag="bcs0", name="bcs0")
                bcs1 = wpool.tile([64, 512], F32, tag="bcs1", name="bcs1")
                nc.gpsimd.partition_broadcast(bcs0[:], inv[0:1, 0:512])
                nc.gpsimd.partition_broadcast(bcs1[:], inv[0:1, 512:1024])
                nc.gpsimd.tensor_mul(otn_sb[p][0:64, qsl], s64a[0:64, :],
                                     bcs0[:])
                oth = wpool.tile([64, 512], BF16, tag="oth", name="oth")
                nc.gpsimd.tensor_mul(oth[:], s64b[0:64, :], bcs1[:])
                # partition-shifting copy into rows 64:128
                nc.sync.dma_start(otn_sb[p][64:128, qsl], oth[:])

        # ---- remaining output chunks (q-tile 3 of pair 3) ----
        for sc in range(12, SC):
            emit_out_chunk(sc)

    nc.compile()
    return nc


def tc_scope(nc):
    """Open the TileContext plus all pools; returns a context manager
    yielding (tc, cpool, ptpool, wpool, pspool)."""
    import contextlib

    @contextlib.contextmanager
    def scope():
        with tile.TileContext(nc) as tc:
            with (
                tc.tile_pool(name="const", bufs=1) as cpool,
                tc.tile_pool(name="pt", bufs=8) as ptpool,
                tc.tile_pool(name="work", bufs=2) as wpool,
                tc.tile_pool(name="ps", bufs=2, space="PSUM") as pspool,
            ):
                yield tc, cpool, ptpool, wpool, pspool

    return scope()


_NC_CACHE = None


def _get_nc():
    global _NC_CACHE
    if _NC_CACHE is None:
        _NC_CACHE = build_nc()
    return _NC_CACHE


def make_in_maps(x, w_qkv, w_proj):
    """Shard full inputs into the 8 per-core input dicts."""
    bf = ml_dtypes.bfloat16
    mask01 = (np.arange(128)[:, None] <= np.arange(512)[None, :]) \
        .astype(bf)
    in_maps = []
    for core in range(N_CORES):
        b, g = core // 2, core % 2
        gsl = slice(GW * g, GW * (g + 1))
        in_maps.append({
            "xT": np.ascontiguousarray(x[b].T).astype(bf),
            "wq": np.ascontiguousarray(w_qkv[:, 0 * C:1 * C][:, gsl]).astype(bf),
            "wk": np.ascontiguousarray(w_qkv[:, 1 * C:2 * C][:, gsl]).astype(bf),
            "wv": np.ascontiguousarray(w_qkv[:, 2 * C:3 * C][:, gsl]).astype(bf),
            "wp": np.ascontiguousarray(w_proj[gsl, :]).astype(bf),
            "mask": mask01,
        })
    return in_maps


def kernel(x, w_qkv, w_proj, b_proj, _profile=False):
    import os
    if not _profile:
        # the NTFF trace path needs modules absent from this image;
        # make sure an inherited BASS_TRACE can't route us into it
        os.environ["BASS_NEVER_TRACE"] = "1"
    else:
        os.environ.pop("BASS_NEVER_TRACE", None)
    x = np.asarray(x, np.float32)
    w_qkv = np.asarray(w_qkv, np.float32)
    w_proj = np.asarray(w_proj, np.float32)
    b_proj = np.asarray(b_proj, np.float32)

    nc = _get_nc()
    in_maps = make_in_maps(x, w_qkv, w_proj)
    res = run_bass_kernel_spmd(nc, in_maps, core_ids=list(range(N_CORES)),
                               trace=_profile)
    partials = [np.asarray(res.results[c]["out"], np.float32)
                for c in range(N_CORES)]
    out = np.empty((B, S, C), np.float32)
    for b in range(B):
        out[b] = partials[2 * b] + partials[2 * b + 1] + b_proj
    if _profile:
        return out, res
    return out


# revision 8
# speedup vs baseline: 1.2845x; 1.2845x over previous
"""Causal multi-head flash-attention block (QKV proj + attention + out proj)
for Trainium2, distributed over 8 NeuronCores.

Sharding: data-parallel over batch (B=4) x tensor-parallel over head groups
(16 heads -> 2 groups of 8). Core c handles batch c//2, head group c%2.
Each core computes a partial output projection (its 8 heads' contribution);
the host sums the two partials per batch and adds the bias.

v3 schedule notes: the attention inner loop is ACT(exp)-bound, and the PE
executes its queue strictly in order, so independent matmul work is
interleaved INTO the attention k-block loop to keep the PE dense:
  - each pair's (st2,st3) QT/KT units fill its own j0/j1, the NEXT pair's
    (st0,st1) units fill j2/j3 (so casts land well before the pair switch),
  - V strips fill pair 0, out-proj chunks fill pair 3's late slots,
  - inputs arrive via 13 grouped multi-chunk DMAs (3D access patterns) so
    the sync queue isn't issue-bound; pair-0 st0/st1 QT/KT runs tile-minor
    over the arriving chunk-pairs (PE dense from ~11us).
Scores/exp are k-block granular: one ACTIVATE per k-block covering both
packed heads, restricted to causally live columns on diagonal blocks.
The softmax-normalize chain is split in two phases and issues its DMAs
from the producer engines' own queues (dens from vector, the oth
partition-shift from gpsimd) so no FIFO head-of-line-blocks another
engine's PE-feeding work; the multiplies run late (deferred one q-tile)
on the DVE after the gpsimd broadcast is long done.

Per-core kernel (all matmuls bf16 operands, fp32 PSUM accumulate):
  - QKV proj from host-pretransposed x^T: Q^T,K^T in [d, s] layout, V in
    [s, d] layout with a ones-column per head (rowsum trick).
  - Scores transposed: ST[k,q] via lhsT=KT-block, rhs=QT; two heads packed
    via PE row tiling (K=64 each, partitions 0:64 / 64:128, one XBUS).
  - softmax without max-subtraction (logits ~ N(0,1)); exp on ACT with the
    1/8 scale folded in; causal 0/1 mask multiply post-exp on diagonal
    blocks; fully-masked blocks skipped.
  - AV: lhsT = V-tile [128, 65] (65th col = ones -> row 64 of PSUM is the
    softmax denominator), rhs = P^T tiles.
  - Normalize: psum row 64 -> partition 0 via tiny DMAs, merged
    reciprocal_approx_fast, gpsimd partition_broadcast, DVE multiplies.
  - Output proj from O^T [head*64+d, s] chunks against w_proj rows;
    partial outputs written bf16 (host sums in f32).
"""

import numpy as np
import ml_dtypes

import concourse.bass as bass
import concourse.bacc as bacc
import concourse.mybir as mybir
import concourse.tile as tile
from concourse.bass_utils import run_bass_kernel_spmd

F32 = mybir.dt.float32
BF16 = mybir.dt.bfloat16
EXP = mybir.ActivationFunctionType.Exp

# Problem constants (hardcoded per contract)
B, S, C = 4, 2048, 1024
NH, D = 16, 64
SCALE = D ** -0.5
N_CORES = 8
HG = NH // 2          # heads per core (head group)
NPAIR = HG // 2       # head pairs per core
CCH = C // 128        # contraction chunks for QKV proj
SC = S // 128         # s-chunks (also k-blocks count)
NQT = S // 512        # q-tiles of 512
GW = C // 2           # group width of qkv output (8 heads * 64)


def build_nc():
    nc = bacc.Bacc("TRN2", target_bir_lowering=False, debug=False)

    xT = nc.dram_tensor("xT", [C, S], BF16, kind="ExternalInput")
    wq = nc.dram_tensor("wq", [C, GW], BF16, kind="ExternalInput")
    wk = nc.dram_tensor("wk", [C, GW], BF16, kind="ExternalInput")
    wv = nc.dram_tensor("wv", [C, GW], BF16, kind="ExternalInput")
    wp = nc.dram_tensor("wp", [GW, C], BF16, kind="ExternalInput")
    mask = nc.dram_tensor("mask", [128, 512], BF16, kind="ExternalInput")
    out = nc.dram_tensor("out", [S, C], BF16, kind="ExternalOutput")

    with tile.TileContext(nc) as tc:
        with (
            tc.tile_pool(name="const", bufs=1) as cpool,
            tc.tile_pool(name="pt", bufs=8) as ptpool,
            tc.tile_pool(name="work", bufs=2) as wpool,
            tc.tile_pool(name="ps", bufs=2, space="PSUM") as pspool,
        ):
            # ---- persistent tiles; grouped input DMAs (3D APs) ----
            # chunk cc of a weight lives at cols [512cc:512(cc+1)];
            # chunk cc of xT at cols [2048cc:2048(cc+1)].
            wqall = cpool.tile([128, GW * CCH], BF16, tag="wqall", name="wqall")
            wkall = cpool.tile([128, GW * CCH], BF16, tag="wkall", name="wkall")
            wvall = cpool.tile([128, GW * CCH], BF16, tag="wvall", name="wvall")
            xtall = cpool.tile([128, S * CCH], BF16, tag="xtall", name="xtall")
            mask_sb = cpool.tile([128, 512], BF16, tag="mask", name="maskt")

            def chunked(t, width):
                # partition dim stays outermost (SBUF AP requirement)
                return t[:, :].rearrange("p (c f) -> p c f", f=width)

            def dram_chunked(t, rows=128):
                return t[:, :].rearrange("(c p) f -> p c f", p=rows)

            nc.sync.dma_start(chunked(wqall, GW), dram_chunked(wq))
            nc.sync.dma_start(chunked(wkall, GW), dram_chunked(wk))
            nc.sync.dma_start(mask_sb[:], mask[:, :])
            # xT in chunk-pairs x column-halves: the st0/st1 ramp chases
            # half-A chunk-pair arrivals
            for P in range(4):
                nc.sync.dma_start(
                    chunked(xtall, S)[:, 2 * P:2 * P + 2, 0:1024],
                    dram_chunked(xT)[:, 2 * P:2 * P + 2, 0:1024])
            for P in range(4):
                nc.sync.dma_start(
                    chunked(xtall, S)[:, 2 * P:2 * P + 2, 1024:2048],
                    dram_chunked(xT)[:, 2 * P:2 * P + 2, 1024:2048])
            nc.sync.dma_start(chunked(wvall, GW), dram_chunked(wv))
            wpall = cpool.tile([128, C * NPAIR], BF16, tag="wpall", name="wpall")
            nc.sync.dma_start(chunked(wpall, C), dram_chunked(wp))

            def xt_c(cc):      # xT chunk cc, [128, S]
                return xtall[:, S * cc:S * (cc + 1)]

            def w_c(wall, cc):  # weight chunk cc, [128, GW]
                return wall[:, GW * cc:GW * (cc + 1)]

            # preload the ACT exp table set while input DMAs run
            actwarm = cpool.tile([1, 8], F32, tag="actwarm", name="actwarm")
            nc.vector.memset(actwarm[:], 0.0)
            nc.scalar.activation(actwarm[:], actwarm[:], EXP)

            qt_sb = [cpool.tile([128, S], BF16, tag=f"qt{p}", name=f"qt{p}")
                     for p in range(NPAIR)]
            kt_sb = [cpool.tile([128, S], BF16, tag=f"kt{p}", name=f"kt{p}")
                     for p in range(NPAIR)]
            otn_sb = [cpool.tile([128, S], BF16, tag=f"otn{p}", name=f"otn{p}")
                      for p in range(NPAIR)]
            vt_sb = [cpool.tile([128, 65 * HG], BF16, tag=f"vt{sc}",
                                name=f"vt{sc}")
                     for sc in range(SC)]

            # ---- QT/KT projection unit: one (pair, st, q|k) tile ----
            def emit_qkt_unit(p, st, which):
                ssl = slice(512 * st, 512 * (st + 1))
                wall, dst = (wqall, qt_sb[p]) if which == 0 else \
                    (wkall, kt_sb[p])
                ps = pspool.tile([128, 512], F32, tag="mm", name="qkps",
                                 bufs=2)
                for cc in range(CCH):
                    nc.tensor.matmul(
                        ps[:], w_c(wall, cc)[:, 128 * p:128 * (p + 1)],
                        xt_c(cc)[:, ssl],
                        start=(cc == 0), stop=(cc == CCH - 1))
                nc.vector.tensor_copy(dst[:, ssl], ps[:])

            # ---- pair-0 st0/st1 ramp: tile-minor over 4 accumulators,
            # cc ascending, chasing the half-A chunk-pair DMAs
            ramp = [(0, 0, "sps"), (0, 1, "sps"), (1, 0, "ot"), (1, 1, "ot")]
            pss = [pspool.tile([128, 512], F32, tag=ptag, name="rampps",
                               bufs=2) for _, _, ptag in ramp]
            for cc in range(CCH):
                for (st, which, _), ps in zip(ramp, pss):
                    wall = wqall if which == 0 else wkall
                    nc.tensor.matmul(
                        ps[:], w_c(wall, cc)[:, 0:128],
                        xt_c(cc)[:, 512 * st:512 * (st + 1)],
                        start=(cc == 0), stop=(cc == CCH - 1))
            for (st, which, _), ps in zip(ramp, pss):
                dst = qt_sb[0] if which == 0 else kt_sb[0]
                nc.vector.tensor_copy(dst[:, 512 * st:512 * (st + 1)], ps[:])

            # ---- V strip: V = x @ wv in [s, d] layout + ones column ----
            def emit_v_strip(sc):
                vt = vt_sb[sc]
                nc.gpsimd.memset(vt[:], 1.0)
                ps = pspool.tile([128, GW], F32, tag="mm", name="vps",
                                 bufs=2)
                for cc in range(CCH):
                    nc.tensor.matmul(
                        ps[:], xt_c(cc)[:, 128 * sc:128 * (sc + 1)],
                        w_c(wvall, cc)[:],
                        start=(cc == 0), stop=(cc == CCH - 1))
                vt_v = vt[:, :].rearrange("p (h d) -> p h d", h=HG)[:, :, 0:64]
                ps_v = ps[:, :].rearrange("p (h d) -> p h d", h=HG)
                nc.vector.tensor_copy(vt_v, ps_v)

            for sc in range(4):
                emit_v_strip(sc)

            # ---- out-proj chunk: out[s-chunk,:] = sum_p OTn_p.T @ wp_p ----
            def emit_out_chunk(sc):
                outst = wpool.tile([128, C], BF16, tag="outst", name="outst")
                for half in range(2):
                    pp = pspool.tile([128, 512], F32, tag="mm", name="pp",
                                     bufs=2)
                    for p in range(NPAIR):
                        nc.tensor.matmul(
                            pp[:], otn_sb[p][:, 128 * sc:128 * (sc + 1)],
                            wpall[:, C * p + 512 * half:
                                  C * p + 512 * (half + 1)],
                            start=(p == 0), stop=(p == NPAIR - 1))
                    nc.vector.tensor_copy(
                        outst[:, 512 * half:512 * (half + 1)], pp[:])
                nc.sync.dma_start(out[128 * sc:128 * (sc + 1), :], outst[:])

            # ---- per head-pair attention, k-block granular ----
            deferred_norm = [None]  # phase-2 closure from the previous j

            def run_deferred():
                if deferred_norm[0] is not None:
                    deferred_norm[0]()
                    deferred_norm[0] = None

            for p in range(NPAIR):
                qt, kt = qt_sb[p], kt_sb[p]
                for j in range(NQT):
                    nkb = 4 * (j + 1)  # causal: only k-blocks 0..nkb-1
                    # filler units for this (p, j) window
                    fillers = []
                    if p == 0 and j < 3:
                        for sc in range(4 * (j + 1), 4 * (j + 2)):
                            fillers.append(lambda sc=sc: emit_v_strip(sc))
                    if j < 2:
                        # own (st2, st3) QT/KT: 2 units per j
                        for which in range(2):
                            fillers.append(
                                lambda st=j + 2, w=which, pp_=p:
                                emit_qkt_unit(pp_, st, w))
                    elif p < 3:
                        # next pair's (st0, st1): 2 units per j
                        for which in range(2):
                            fillers.append(
                                lambda st=j - 2, w=which, pp_=p + 1:
                                emit_qkt_unit(pp_, st, w))
                    fill_at = {}
                    for fi in range(len(fillers)):
                        g = min(nkb - 1, (fi * nkb) // max(1, len(fillers)))
                        fill_at.setdefault(g, []).append(fillers[fi])
                    if p == 3 and j >= 1:
                        # out-proj chunks of q-tile j-1 in the last two
                        # slots (their otn inputs come from the previous
                        # normalize; placing them early would stall the
                        # in-order PE)
                        for i, sc in enumerate(range(4 * (j - 1), 4 * j)):
                            fill_at.setdefault(nkb - 2 + (i % 2), []).append(
                                lambda sc=sc: emit_out_chunk(sc))

                    ot = [pspool.tile([65, 512], F32, tag="ot", name="ot",
                                      bufs=2) for _ in range(2)]

                    def emit_av(kb, pt, j=j, nkb=nkb, ot=ot, p=p):
                        o = 128 * (kb - 4 * j) if kb >= 4 * j else 0
                        for h in range(2):
                            nc.tensor.matmul(
                                ot[h][:, o:512],
                                vt_sb[kb][:, 65 * (2 * p + h):
                                          65 * (2 * p + h) + 65],
                                pt[:, 512 * h + o:512 * (h + 1)],
                                start=(kb == 0), stop=(kb == nkb - 1))

                    pending = []
                    for kb in range(nkb):
                        diag = kb >= 4 * j
                        o = 128 * (kb - 4 * j) if diag else 0
                        # scores (transposed), both heads packed side by
                        # side in one psum tile: h at cols [512h+o:512h+512]
                        sp = pspool.tile([128, 1024], F32, tag="sps",
                                         name="sps", bufs=2)
                        for h in range(2):
                            hsl = slice(64 * h, 64 * (h + 1))
                            nc.tensor.matmul(
                                sp[:, 512 * h + o:512 * (h + 1)],
                                kt[hsl, 128 * kb:128 * (kb + 1)],
                                qt[hsl, 512 * j + o:512 * (j + 1)],
                                start=True, stop=True)
                        pt = ptpool.tile([128, 1024], BF16, tag="pt",
                                         name="pt")
                        if o > 0:
                            # one ACTIVATE over both heads' live columns
                            spv = sp[:, :].rearrange(
                                "p (h f) -> p h f", h=2)[:, :, o:512]
                            ptv = pt[:, :].rearrange(
                                "p (h f) -> p h f", h=2)[:, :, o:512]
                            nc.scalar.activation(ptv, spv, EXP, scale=SCALE)
                        else:
                            nc.scalar.activation(pt[:], sp[:], EXP,
                                                 scale=SCALE)
                        if diag:
                            for h in range(2):
                                csl = slice(512 * h + o, 512 * (h + 1))
                                nc.vector.tensor_mul(
                                    pt[:, csl], pt[:, csl],
                                    mask_sb[:, 0:512 - o])
                        pending.append((kb, pt))
                        if len(pending) > 4:
                            emit_av(*pending.pop(0))
                        if kb == 1:
                            run_deferred()
                        for f in fill_at.get(kb, []):
                            f()
                    for item in pending:
                        emit_av(*item)

                    # ---- normalize phase 1: numerators to SBUF (releases
                    # psum), denominators to partition 0 via tiny
                    # vector-issued DMAs, merged reciprocal, broadcasts.
                    qsl = slice(512 * j, 512 * (j + 1))
                    s64a = wpool.tile([65, 512], F32, tag="s64a", name="s64a")
                    s64b = wpool.tile([65, 512], F32, tag="s64b", name="s64b")
                    nc.vector.tensor_copy(s64a[:, :], ot[0][:, :])
                    nc.vector.tensor_copy(s64b[:, :], ot[1][:, :])
                    dens = wpool.tile([1, 1024], F32, tag="dens", name="dens")
                    nc.gpsimd.dma_start(dens[0:1, 0:512], s64a[64:65, :])
                    nc.gpsimd.dma_start(dens[0:1, 512:1024], s64b[64:65, :])
                    inv = wpool.tile([1, 1024], F32, tag="inv", name="inv")
                    nc.vector.reciprocal_approx_fast(inv[0:1, :],
                                                     dens[0:1, :])
                    bcs0 = wpool.tile([64, 512], F32, tag="bcs0", name="bcs0")
                    bcs1 = wpool.tile([64, 512], F32, tag="bcs1", name="bcs1")
                    nc.gpsimd.partition_broadcast(bcs0[:], inv[0:1, 0:512])
                    nc.gpsimd.partition_broadcast(bcs1[:], inv[0:1, 512:1024])

                    # ---- phase 2 (deferred one q-tile): multiplies on DVE
                    # (broadcast long done by then), partition-shifting oth
                    # copy issued from the gpsimd queue.
                    def phase2(p=p, qsl=qsl, s64a=s64a, s64b=s64b,
                               bcs0=bcs0, bcs1=bcs1):
                        nc.vector.tensor_mul(otn_sb[p][0:64, qsl],
                                             s64a[0:64, :], bcs0[:])
                        oth = wpool.tile([64, 512], BF16, tag="oth",
                                         name="oth")
                        nc.vector.tensor_mul(oth[:], s64b[0:64, :], bcs1[:])
                        nc.gpsimd.dma_start(otn_sb[p][64:128, qsl], oth[:])

                    deferred_norm[0] = phase2

            # ---- tail: last normalize phase 2, remaining output chunks ----
            run_deferred()
            for sc in range(12, SC):
                emit_out_chunk(sc)

    nc.compile()
    return nc


_NC_CACHE = None


def _get_nc():
    global _NC_CACHE
    if _NC_CACHE is None:
        _NC_CACHE = build_nc()
    return _NC_CACHE


def make_in_maps(x, w_qkv, w_proj):
    """Shard full inputs into the 8 per-core input dicts."""
    bf = ml_dtypes.bfloat16
    mask01 = (np.arange(128)[:, None] <= np.arange(512)[None, :]) \
        .astype(bf)
    in_maps = []
    for core in range(N_CORES):
        b, g = core // 2, core % 2
        gsl = slice(GW * g, GW * (g + 1))
        in_maps.append({
            "xT": np.ascontiguousarray(x[b].T).astype(bf),
            "wq": np.ascontiguousarray(w_qkv[:, 0 * C:1 * C][:, gsl]).astype(bf),
            "wk": np.ascontiguousarray(w_qkv[:, 1 * C:2 * C][:, gsl]).astype(bf),
            "wv": np.ascontiguousarray(w_qkv[:, 2 * C:3 * C][:, gsl]).astype(bf),
            "wp": np.ascontiguousarray(w_proj[gsl, :]).astype(bf),
            "mask": mask01,
        })
    return in_maps


def kernel(x, w_qkv, w_proj, b_proj, _profile=False):
    import os
    if not _profile:
        # the NTFF trace path needs modules absent from this image;
        # make sure an inherited BASS_TRACE can't route us into it
        os.environ["BASS_NEVER_TRACE"] = "1"
    else:
        os.environ.pop("BASS_NEVER_TRACE", None)
    x = np.asarray(x, np.float32)
    w_qkv = np.asarray(w_qkv, np.float32)
    w_proj = np.asarray(w_proj, np.float32)
    b_proj = np.asarray(b_proj, np.float32)

    nc = _get_nc()
    in_maps = make_in_maps(x, w_qkv, w_proj)
    res = run_bass_kernel_spmd(nc, in_maps, core_ids=list(range(N_CORES)),
                               trace=_profile)
    partials = [np.asarray(res.results[c]["out"], np.float32)
                for c in range(N_CORES)]
    out = np.empty((B, S, C), np.float32)
    for b in range(B):
        out[b] = partials[2 * b] + partials[2 * b + 1] + b_proj
    if _profile:
        return out, res
    return out


# revision 10
# speedup vs baseline: 1.2880x; 1.0028x over previous
"""Causal multi-head flash-attention block (QKV proj + attention + out proj)
for Trainium2, distributed over 8 NeuronCores.

Sharding: data-parallel over batch (B=4) x tensor-parallel over head groups
(16 heads -> 2 groups of 8). Core c handles batch c//2, head group c%2.
Each core computes a partial output projection (its 8 heads' contribution);
the host sums the two partials per batch and adds the bias.

v3 schedule notes: the attention inner loop is ACT(exp)-bound, and the PE
executes its queue strictly in order, so independent matmul work is
interleaved INTO the attention k-block loop to keep the PE dense:
  - each pair's (st2,st3) QT/KT units fill its own j0/j1, the NEXT pair's
    (st0,st1) units fill j2/j3 (so casts land well before the pair switch),
  - V strips fill pair 0, out-proj chunks fill pair 3's late slots,
  - inputs arrive via 13 grouped multi-chunk DMAs (3D access patterns) so
    the sync queue isn't issue-bound; pair-0 st0/st1 QT/KT runs tile-minor
    over the arriving chunk-pairs (PE dense from ~11us).
Scores/exp are k-block granular: one ACTIVATE per k-block covering both
packed heads, restricted to causally live columns on diagonal blocks.
The softmax-normalize chain is split in two phases and issues its DMAs
from the producer engines' own queues (dens from vector, the oth
partition-shift from gpsimd) so no FIFO head-of-line-blocks another
engine's PE-feeding work; the multiplies run late (deferred one q-tile)
on the DVE after the gpsimd broadcast is long done.

Per-core kernel (all matmuls bf16 operands, fp32 PSUM accumulate):
  - QKV proj from host-pretransposed x^T: Q^T,K^T in [d, s] layout, V in
    [s, d] layout with a ones-column per head (rowsum trick).
  - Scores transposed: ST[k,q] via lhsT=KT-block, rhs=QT; two heads packed
    via PE row tiling (K=64 each, partitions 0:64 / 64:128, one XBUS).
  - softmax without max-subtraction (logits ~ N(0,1)); exp on ACT with the
    1/8 scale folded in; causal 0/1 mask multiply post-exp on diagonal
    blocks; fully-masked blocks skipped.
  - AV: lhsT = V-tile [128, 65] (65th col = ones -> row 64 of PSUM is the
    softmax denominator), rhs = P^T tiles.
  - Normalize: psum row 64 -> partition 0 via tiny DMAs, merged
    reciprocal_approx_fast, gpsimd partition_broadcast, DVE multiplies.
  - Output proj from O^T [head*64+d, s] chunks against w_proj rows;
    partial outputs written bf16 (host sums in f32).
"""

import numpy as np
import ml_dtypes

import concourse.bass as bass
import concourse.bacc as bacc
import concourse.mybir as mybir
import concourse.tile as tile
from concourse.bass_utils import run_bass_kernel_spmd

F32 = mybir.dt.float32
BF16 = mybir.dt.bfloat16
EXP = mybir.ActivationFunctionType.Exp

# Problem constants (hardcoded per contract)
B, S, C = 4, 2048, 1024
NH, D = 16, 64
SCALE = D ** -0.5
N_CORES = 8
HG = NH // 2          # heads per core (head group)
NPAIR = HG // 2       # head pairs per core
CCH = C // 128        # contraction chunks for QKV proj
SC = S // 128         # s-chunks (also k-blocks count)
NQT = S // 512        # q-tiles of 512
GW = C // 2           # group width of qkv output (8 heads * 64)


def build_nc():
    nc = bacc.Bacc("TRN2", target_bir_lowering=False, debug=False)

    xT = nc.dram_tensor("xT", [C, S], BF16, kind="ExternalInput")
    wq = nc.dram_tensor("wq", [C, GW], BF16, kind="ExternalInput")
    wk = nc.dram_tensor("wk", [C, GW], BF16, kind="ExternalInput")
    wv = nc.dram_tensor("wv", [C, GW], BF16, kind="ExternalInput")
    wp = nc.dram_tensor("wp", [GW, C], BF16, kind="ExternalInput")
    mask = nc.dram_tensor("mask", [128, 512], BF16, kind="ExternalInput")
    out = nc.dram_tensor("out", [S, C], BF16, kind="ExternalOutput")

    with tile.TileContext(nc) as tc:
        with (
            tc.tile_pool(name="const", bufs=1) as cpool,
            tc.tile_pool(name="pt", bufs=8) as ptpool,
            tc.tile_pool(name="work", bufs=2) as wpool,
            tc.tile_pool(name="ps", bufs=2, space="PSUM") as pspool,
        ):
            # ---- persistent tiles; grouped input DMAs (3D APs) ----
            # chunk cc of a weight lives at cols [512cc:512(cc+1)];
            # chunk cc of xT at cols [2048cc:2048(cc+1)].
            wqall = cpool.tile([128, GW * CCH], BF16, tag="wqall", name="wqall")
            wkall = cpool.tile([128, GW * CCH], BF16, tag="wkall", name="wkall")
            wvall = cpool.tile([128, GW * CCH], BF16, tag="wvall", name="wvall")
            xtall = cpool.tile([128, S * CCH], BF16, tag="xtall", name="xtall")
            mask_sb = cpool.tile([128, 512], BF16, tag="mask", name="maskt")

            # per-chunk DMAs: dram reads stay sequential (a grouped 3D AP
            # with partition-outer ordering turns into 2KB strided bursts
            # at ~51 GB/s -- measured). Arrival order: wq/wk/mask, xT
            # half-A (the st0/st1 ramp chases these), wv, xT half-B, wp.
            for cc in range(CCH):
                nc.sync.dma_start(wqall[:, GW * cc:GW * (cc + 1)],
                                  wq[128 * cc:128 * (cc + 1), :])
            for cc in range(CCH):
                nc.sync.dma_start(wkall[:, GW * cc:GW * (cc + 1)],
                                  wk[128 * cc:128 * (cc + 1), :])
            nc.sync.dma_start(mask_sb[:], mask[:, :])
            for cc in range(CCH):
                nc.sync.dma_start(xtall[:, S * cc:S * cc + 1024],
                                  xT[128 * cc:128 * (cc + 1), 0:1024])
            for cc in range(CCH):
                nc.sync.dma_start(wvall[:, GW * cc:GW * (cc + 1)],
                                  wv[128 * cc:128 * (cc + 1), :])
            for cc in range(CCH):
                nc.sync.dma_start(xtall[:, S * cc + 1024:S * (cc + 1)],
                                  xT[128 * cc:128 * (cc + 1), 1024:2048])
            wpall = cpool.tile([128, C * NPAIR], BF16, tag="wpall", name="wpall")
            for p in range(NPAIR):
                nc.sync.dma_start(wpall[:, C * p:C * (p + 1)],
                                  wp[128 * p:128 * (p + 1), :])

            def xt_c(cc):      # xT chunk cc, [128, S]
                return xtall[:, S * cc:S * (cc + 1)]

            def w_c(wall, cc):  # weight chunk cc, [128, GW]
                return wall[:, GW * cc:GW * (cc + 1)]

            # preload the ACT exp table set while input DMAs run
            actwarm = cpool.tile([1, 8], F32, tag="actwarm", name="actwarm")
            nc.vector.memset(actwarm[:], 0.0)
            nc.scalar.activation(actwarm[:], actwarm[:], EXP)

            qt_sb = [cpool.tile([128, S], BF16, tag=f"qt{p}", name=f"qt{p}")
                     for p in range(NPAIR)]
            kt_sb = [cpool.tile([128, S], BF16, tag=f"kt{p}", name=f"kt{p}")
                     for p in range(NPAIR)]
            otn_sb = [cpool.tile([128, S], BF16, tag=f"otn{p}", name=f"otn{p}")
                      for p in range(NPAIR)]
            vt_sb = [cpool.tile([128, 65 * HG], BF16, tag=f"vt{sc}",
                                name=f"vt{sc}")
                     for sc in range(SC)]

            # ---- QT/KT projection unit: one (pair, st, q|k) tile ----
            def emit_qkt_unit(p, st, which):
                ssl = slice(512 * st, 512 * (st + 1))
                wall, dst = (wqall, qt_sb[p]) if which == 0 else \
                    (wkall, kt_sb[p])
                ps = pspool.tile([128, 512], F32, tag="mm", name="qkps",
                                 bufs=2)
                for cc in range(CCH):
                    nc.tensor.matmul(
                        ps[:], w_c(wall, cc)[:, 128 * p:128 * (p + 1)],
                        xt_c(cc)[:, ssl],
                        start=(cc == 0), stop=(cc == CCH - 1))
                nc.vector.tensor_copy(dst[:, ssl], ps[:])

            # ---- pair-0 st0/st1 ramp: tile-minor over 4 accumulators,
            # cc ascending, chasing the half-A chunk-pair DMAs
            ramp = [(0, 0, "sps"), (0, 1, "sps"), (1, 0, "ot"), (1, 1, "ot")]
            pss = [pspool.tile([128, 512], F32, tag=ptag, name="rampps",
                               bufs=2) for _, _, ptag in ramp]
            for cc in range(CCH):
                for (st, which, _), ps in zip(ramp, pss):
                    wall = wqall if which == 0 else wkall
                    nc.tensor.matmul(
                        ps[:], w_c(wall, cc)[:, 0:128],
                        xt_c(cc)[:, 512 * st:512 * (st + 1)],
                        start=(cc == 0), stop=(cc == CCH - 1))
            for (st, which, _), ps in zip(ramp, pss):
                dst = qt_sb[0] if which == 0 else kt_sb[0]
                nc.vector.tensor_copy(dst[:, 512 * st:512 * (st + 1)], ps[:])

            # ---- V strip: V = x @ wv in [s, d] layout + ones column ----
            def emit_v_strip(sc):
                vt = vt_sb[sc]
                nc.gpsimd.memset(vt[:], 1.0)
                ps = pspool.tile([128, GW], F32, tag="mm", name="vps",
                                 bufs=2)
                for cc in range(CCH):
                    nc.tensor.matmul(
                        ps[:], xt_c(cc)[:, 128 * sc:128 * (sc + 1)],
                        w_c(wvall, cc)[:],
                        start=(cc == 0), stop=(cc == CCH - 1))
                vt_v = vt[:, :].rearrange("p (h d) -> p h d", h=HG)[:, :, 0:64]
                ps_v = ps[:, :].rearrange("p (h d) -> p h d", h=HG)
                nc.vector.tensor_copy(vt_v, ps_v)

            for sc in range(4):
                emit_v_strip(sc)

            # ---- out-proj chunk: out[s-chunk,:] = sum_p OTn_p.T @ wp_p ----
            def emit_out_chunk(sc):
                outst = wpool.tile([128, C], BF16, tag="outst", name="outst")
                for half in range(2):
                    pp = pspool.tile([128, 512], F32, tag="mm", name="pp",
                                     bufs=2)
                    for p in range(NPAIR):
                        nc.tensor.matmul(
                            pp[:], otn_sb[p][:, 128 * sc:128 * (sc + 1)],
                            wpall[:, C * p + 512 * half:
                                  C * p + 512 * (half + 1)],
                            start=(p == 0), stop=(p == NPAIR - 1))
                    nc.vector.tensor_copy(
                        outst[:, 512 * half:512 * (half + 1)], pp[:])
                nc.sync.dma_start(out[128 * sc:128 * (sc + 1), :], outst[:])

            # ---- per head-pair attention, k-block granular ----
            deferred_norm = [None]  # phase-2 closure from the previous j

            def run_deferred():
                if deferred_norm[0] is not None:
                    deferred_norm[0]()
                    deferred_norm[0] = None

            for p in range(NPAIR):
                qt, kt = qt_sb[p], kt_sb[p]
                for j in range(NQT):
                    nkb = 4 * (j + 1)  # causal: only k-blocks 0..nkb-1
                    # filler units for this (p, j) window
                    fillers = []
                    if p == 0 and j < 3:
                        for sc in range(4 * (j + 1), 4 * (j + 2)):
                            fillers.append(lambda sc=sc: emit_v_strip(sc))
                    if j < 2:
                        # own (st2, st3) QT/KT: 2 units per j
                        for which in range(2):
                            fillers.append(
                                lambda st=j + 2, w=which, pp_=p:
                                emit_qkt_unit(pp_, st, w))
                    elif p < 3:
                        # next pair's (st0, st1): 2 units per j
                        for which in range(2):
                            fillers.append(
                                lambda st=j - 2, w=which, pp_=p + 1:
                                emit_qkt_unit(pp_, st, w))
                    fill_at = {}
                    for fi in range(len(fillers)):
                        g = min(nkb - 1, (fi * nkb) // max(1, len(fillers)))
                        fill_at.setdefault(g, []).append(fillers[fi])
                    if p == 3 and j >= 1:
                        # out-proj chunks of q-tile j-1 in the last two
                        # slots (their otn inputs come from the previous
                        # normalize; placing them early would stall the
                        # in-order PE)
                        for i, sc in enumerate(range(4 * (j - 1), 4 * j)):
                            fill_at.setdefault(nkb - 2 + (i % 2), []).append(
                                lambda sc=sc: emit_out_chunk(sc))

                    ot = [pspool.tile([65, 512], F32, tag="ot", name="ot",
                                      bufs=2) for _ in range(2)]

                    def emit_av(kb, pt, j=j, nkb=nkb, ot=ot, p=p):
                        o = 128 * (kb - 4 * j) if kb >= 4 * j else 0
                        for h in range(2):
                            nc.tensor.matmul(
                                ot[h][:, o:512],
                                vt_sb[kb][:, 65 * (2 * p + h):
                                          65 * (2 * p + h) + 65],
                                pt[:, 512 * h + o:512 * (h + 1)],
                                start=(kb == 0), stop=(kb == nkb - 1))

                    pending = []
                    for kb in range(nkb):
                        diag = kb >= 4 * j
                        o = 128 * (kb - 4 * j) if diag else 0
                        # scores (transposed), both heads packed side by
                        # side in one psum tile: h at cols [512h+o:512h+512]
                        sp = pspool.tile([128, 1024], F32, tag="sps",
                                         name="sps", bufs=2)
                        for h in range(2):
                            hsl = slice(64 * h, 64 * (h + 1))
                            nc.tensor.matmul(
                                sp[:, 512 * h + o:512 * (h + 1)],
                                kt[hsl, 128 * kb:128 * (kb + 1)],
                                qt[hsl, 512 * j + o:512 * (j + 1)],
                                start=True, stop=True)
                        pt = ptpool.tile([128, 1024], BF16, tag="pt",
                                         name="pt")
                        if o > 0:
                            # one ACTIVATE over both heads' live columns
                            spv = sp[:, :].rearrange(
                                "p (h f) -> p h f", h=2)[:, :, o:512]
                            ptv = pt[:, :].rearrange(
                                "p (h f) -> p h f", h=2)[:, :, o:512]
                            nc.scalar.activation(ptv, spv, EXP, scale=SCALE)
                        else:
                            nc.scalar.activation(pt[:], sp[:], EXP,
                                                 scale=SCALE)
                        if diag:
                            for h in range(2):
                                csl = slice(512 * h + o, 512 * (h + 1))
                                nc.vector.tensor_mul(
                                    pt[:, csl], pt[:, csl],
                                    mask_sb[:, 0:512 - o])
                        pending.append((kb, pt))
                        if len(pending) > 4:
                            emit_av(*pending.pop(0))
                        if kb == 1:
                            run_deferred()
                        for f in fill_at.get(kb, []):
                            f()
                    for item in pending:
                        emit_av(*item)

                    # ---- normalize phase 1: numerators to SBUF (releases
                    # psum), denominators to partition 0 via tiny
                    # vector-issued DMAs, merged reciprocal, broadcasts.
                    qsl = slice(512 * j, 512 * (j + 1))
                    s64a = wpool.tile([65, 512], F32, tag="s64a", name="s64a")
                    s64b = wpool.tile([65, 512], F32, tag="s64b", name="s64b")
                    nc.vector.tensor_copy(s64a[:, :], ot[0][:, :])
                    nc.vector.tensor_copy(s64b[:, :], ot[1][:, :])
                    dens = wpool.tile([1, 1024], F32, tag="dens", name="dens")
                    nc.gpsimd.dma_start(dens[0:1, 0:512], s64a[64:65, :])
                    nc.gpsimd.dma_start(dens[0:1, 512:1024], s64b[64:65, :])
                    inv = wpool.tile([1, 1024], F32, tag="inv", name="inv")
                    nc.vector.reciprocal_approx_fast(inv[0:1, :],
                                                     dens[0:1, :])
                    bcs0 = wpool.tile([64, 512], F32, tag="bcs0", name="bcs0")
                    bcs1 = wpool.tile([64, 512], F32, tag="bcs1", name="bcs1")
                    nc.gpsimd.partition_broadcast(bcs0[:], inv[0:1, 0:512])
                    nc.gpsimd.partition_broadcast(bcs1[:], inv[0:1, 512:1024])

                    # ---- phase 2 (deferred one q-tile): multiplies on DVE
                    # (broadcast long done by then), partition-shifting oth
                    # copy issued from the gpsimd queue.
                    def phase2(p=p, qsl=qsl, s64a=s64a, s64b=s64b,
                               bcs0=bcs0, bcs1=bcs1):
                        nc.vector.tensor_mul(otn_sb[p][0:64, qsl],
                                             s64a[0:64, :], bcs0[:])
                        oth = wpool.tile([64, 512], BF16, tag="oth",
                                         name="oth")
                        nc.vector.tensor_mul(oth[:], s64b[0:64, :], bcs1[:])
                        nc.gpsimd.dma_start(otn_sb[p][64:128, qsl], oth[:])

                    deferred_norm[0] = phase2

            # ---- tail: chunks 12-14 pre-accumulate pairs 0-2 while the
            # last normalize chain (otn[3] q-tile 3) is still in flight,
            # then finish with pair 3's contribution; chunk 15 runs whole.
            tail_pps = {}
            for sc, ptag in ((12, "mm"), (13, "sps"), (14, "ot")):
                pps = []
                for half in range(2):
                    pp = pspool.tile([128, 512], F32, tag=ptag, name="pp",
                                     bufs=2)
                    for p in range(3):
                        nc.tensor.matmul(
                            pp[:], otn_sb[p][:, 128 * sc:128 * (sc + 1)],
                            wpall[:, C * p + 512 * half:
                                  C * p + 512 * (half + 1)],
                            start=(p == 0), stop=False)
                    pps.append(pp)
                tail_pps[sc] = pps
            run_deferred()
            for sc in (12, 13, 14):
                outst = wpool.tile([128, C], BF16, tag="outst", name="outst")
                for half, pp in enumerate(tail_pps[sc]):
                    nc.tensor.matmul(
                        pp[:], otn_sb[3][:, 128 * sc:128 * (sc + 1)],
                        wpall[:, C * 3 + 512 * half:C * 3 + 512 * (half + 1)],
                        start=False, stop=True)
                    nc.vector.tensor_copy(
                        outst[:, 512 * half:512 * (half + 1)], pp[:])
                nc.sync.dma_start(out[128 * sc:128 * (sc + 1), :], outst[:])
            emit_out_chunk(15)

    nc.compile()
    return nc


_NC_CACHE = None


def _get_nc():
    global _NC_CACHE
    if _NC_CACHE is None:
        _NC_CACHE = build_nc()
    return _NC_CACHE


def make_in_maps(x, w_qkv, w_proj):
    """Shard full inputs into the 8 per-core input dicts."""
    bf = ml_dtypes.bfloat16
    mask01 = (np.arange(128)[:, None] <= np.arange(512)[None, :]) \
        .astype(bf)
    in_maps = []
    for core in range(N_CORES):
        b, g = core // 2, core % 2
        gsl = slice(GW * g, GW * (g + 1))
        in_maps.append({
            "xT": np.ascontiguousarray(x[b].T).astype(bf),
            "wq": np.ascontiguousarray(w_qkv[:, 0 * C:1 * C][:, gsl]).astype(bf),
            "wk": np.ascontiguousarray(w_qkv[:, 1 * C:2 * C][:, gsl]).astype(bf),
            "wv": np.ascontiguousarray(w_qkv[:, 2 * C:3 * C][:, gsl]).astype(bf),
            "wp": np.ascontiguousarray(w_proj[gsl, :]).astype(bf),
            "mask": mask01,
        })
    return in_maps


def kernel(x, w_qkv, w_proj, b_proj, _profile=False):
    import os
    if not _profile:
        # the NTFF trace path needs modules absent from this image;
        # make sure an inherited BASS_TRACE can't route us into it
        os.environ["BASS_NEVER_TRACE"] = "1"
    else:
        os.environ.pop("BASS_NEVER_TRACE", None)
    x = np.asarray(x, np.float32)
    w_qkv = np.asarray(w_qkv, np.float32)
    w_proj = np.asarray(w_proj, np.float32)
    b_proj = np.asarray(b_proj, np.float32)

    nc = _get_nc()
    in_maps = make_in_maps(x, w_qkv, w_proj)
    res = run_bass_kernel_spmd(nc, in_maps, core_ids=list(range(N_CORES)),
                               trace=_profile)
    partials = [np.asarray(res.results[c]["out"], np.float32)
                for c in range(N_CORES)]
    out = np.empty((B, S, C), np.float32)
    for b in range(B):
        out[b] = partials[2 * b] + partials[2 * b + 1] + b_proj
    if _profile:
        return out, res
    return out


# revision 11
# speedup vs baseline: 1.3219x; 1.0263x over previous
"""Causal multi-head flash-attention block (QKV proj + attention + out proj)
for Trainium2, distributed over 8 NeuronCores.

Sharding: data-parallel over batch (B=4) x tensor-parallel over head groups
(16 heads -> 2 groups of 8). Core c handles batch c//2, head group c%2.
Each core computes a partial output projection (its 8 heads' contribution);
the host sums the two partials per batch and adds the bias.

v3 schedule notes: the attention inner loop is ACT(exp)-bound, and the PE
executes its queue strictly in order, so independent matmul work is
interleaved INTO the attention k-block loop to keep the PE dense:
  - each pair's (st2,st3) QT/KT units fill its own j0/j1, the NEXT pair's
    (st0,st1) units fill j2/j3 (so casts land well before the pair switch),
  - V strips fill pair 0, out-proj chunks fill pair 3's late slots,
  - inputs arrive via 13 grouped multi-chunk DMAs (3D access patterns) so
    the sync queue isn't issue-bound; pair-0 st0/st1 QT/KT runs tile-minor
    over the arriving chunk-pairs (PE dense from ~11us).
Scores/exp are k-block granular: one ACTIVATE per k-block covering both
packed heads, restricted to causally live columns on diagonal blocks.
The softmax-normalize chain is split in two phases and issues its DMAs
from the producer engines' own queues (dens from vector, the oth
partition-shift from gpsimd) so no FIFO head-of-line-blocks another
engine's PE-feeding work; the multiplies run late (deferred one q-tile)
on the DVE after the gpsimd broadcast is long done.

Per-core kernel (all matmuls bf16 operands, fp32 PSUM accumulate):
  - QKV proj from host-pretransposed x^T: Q^T,K^T in [d, s] layout, V in
    [s, d] layout with a ones-column per head (rowsum trick).
  - Scores transposed: ST[k,q] via lhsT=KT-block, rhs=QT; two heads packed
    via PE row tiling (K=64 each, partitions 0:64 / 64:128, one XBUS).
  - softmax without max-subtraction (logits ~ N(0,1)); exp on ACT with the
    1/8 scale folded in; causal 0/1 mask multiply post-exp on diagonal
    blocks; fully-masked blocks skipped.
  - AV: lhsT = V-tile [128, 65] (65th col = ones -> row 64 of PSUM is the
    softmax denominator), rhs = P^T tiles.
  - Normalize: psum row 64 -> partition 0 via tiny DMAs, merged
    reciprocal_approx_fast, gpsimd partition_broadcast, DVE multiplies.
  - Output proj from O^T [head*64+d, s] chunks against w_proj rows;
    partial outputs written bf16 (host sums in f32).
"""

import numpy as np
import ml_dtypes

import concourse.bass as bass
import concourse.bacc as bacc
import concourse.mybir as mybir
import concourse.tile as tile
from concourse.bass_utils import run_bass_kernel_spmd

F32 = mybir.dt.float32
BF16 = mybir.dt.bfloat16
EXP = mybir.ActivationFunctionType.Exp

# Problem constants (hardcoded per contract)
B, S, C = 4, 2048, 1024
NH, D = 16, 64
SCALE = D ** -0.5
N_CORES = 8
HG = NH // 2          # heads per core (head group)
NPAIR = HG // 2       # head pairs per core
CCH = C // 128        # contraction chunks for QKV proj
SC = S // 128         # s-chunks (also k-blocks count)
NQT = S // 512        # q-tiles of 512
GW = C // 2           # group width of qkv output (8 heads * 64)


def build_nc():
    nc = bacc.Bacc("TRN2", target_bir_lowering=False, debug=False)

    xT = nc.dram_tensor("xT", [C, S], BF16, kind="ExternalInput")
    wq = nc.dram_tensor("wq", [C, GW], BF16, kind="ExternalInput")
    wk = nc.dram_tensor("wk", [C, GW], BF16, kind="ExternalInput")
    wv = nc.dram_tensor("wv", [C, GW], BF16, kind="ExternalInput")
    wp = nc.dram_tensor("wp", [GW, C], BF16, kind="ExternalInput")
    mask = nc.dram_tensor("mask", [128, 512], BF16, kind="ExternalInput")
    out = nc.dram_tensor("out", [S, C], BF16, kind="ExternalOutput")

    with tile.TileContext(nc) as tc:
        with (
            tc.tile_pool(name="const", bufs=1) as cpool,
            tc.tile_pool(name="pt", bufs=8) as ptpool,
            tc.tile_pool(name="work", bufs=2) as wpool,
            tc.tile_pool(name="ps", bufs=2, space="PSUM") as pspool,
        ):
            # ---- persistent tiles; grouped input DMAs (3D APs) ----
            # chunk cc of a weight lives at cols [512cc:512(cc+1)];
            # chunk cc of xT at cols [2048cc:2048(cc+1)].
            wqall = cpool.tile([128, GW * CCH], BF16, tag="wqall", name="wqall")
            wkall = cpool.tile([128, GW * CCH], BF16, tag="wkall", name="wkall")
            wvall = cpool.tile([128, GW * CCH], BF16, tag="wvall", name="wvall")
            xtall = cpool.tile([128, S * CCH], BF16, tag="xtall", name="xtall")
            mask_sb = cpool.tile([128, 512], BF16, tag="mask", name="maskt")

            # per-chunk DMAs: dram reads stay sequential (a grouped 3D AP
            # with partition-outer ordering turns into 2KB strided bursts
            # at ~51 GB/s -- measured). Arrival order: wq/wk/mask, xT
            # half-A (the st0/st1 ramp chases these), wv, xT half-B, wp.
            # issue from three engine queues in parallel (the ~0.65us
            # per-issue cost on one queue would gate the ramp): wq from
            # scalar, wk from gpsimd, xT/wv/wp/mask from sync
            for cc in range(CCH):
                nc.scalar.dma_start(wqall[:, GW * cc:GW * (cc + 1)],
                                    wq[128 * cc:128 * (cc + 1), :])
            for cc in range(CCH):
                nc.gpsimd.dma_start(wkall[:, GW * cc:GW * (cc + 1)],
                                    wk[128 * cc:128 * (cc + 1), :])
            nc.sync.dma_start(mask_sb[:], mask[:, :])
            for cc in range(CCH):
                nc.sync.dma_start(xtall[:, S * cc:S * cc + 1024],
                                  xT[128 * cc:128 * (cc + 1), 0:1024])
            for cc in range(CCH):
                nc.sync.dma_start(wvall[:, GW * cc:GW * (cc + 1)],
                                  wv[128 * cc:128 * (cc + 1), :])
            for cc in range(CCH):
                nc.sync.dma_start(xtall[:, S * cc + 1024:S * (cc + 1)],
                                  xT[128 * cc:128 * (cc + 1), 1024:2048])
            wpall = cpool.tile([128, C * NPAIR], BF16, tag="wpall", name="wpall")
            for p in range(NPAIR):
                nc.sync.dma_start(wpall[:, C * p:C * (p + 1)],
                                  wp[128 * p:128 * (p + 1), :])

            def xt_c(cc):      # xT chunk cc, [128, S]
                return xtall[:, S * cc:S * (cc + 1)]

            def w_c(wall, cc):  # weight chunk cc, [128, GW]
                return wall[:, GW * cc:GW * (cc + 1)]

            # preload the ACT exp table set while input DMAs run
            actwarm = cpool.tile([1, 8], F32, tag="actwarm", name="actwarm")
            nc.vector.memset(actwarm[:], 0.0)
            nc.scalar.activation(actwarm[:], actwarm[:], EXP)

            qt_sb = [cpool.tile([128, S], BF16, tag=f"qt{p}", name=f"qt{p}")
                     for p in range(NPAIR)]
            kt_sb = [cpool.tile([128, S], BF16, tag=f"kt{p}", name=f"kt{p}")
                     for p in range(NPAIR)]
            otn_sb = [cpool.tile([128, S], BF16, tag=f"otn{p}", name=f"otn{p}")
                      for p in range(NPAIR)]
            vt_sb = [cpool.tile([128, 65 * HG], BF16, tag=f"vt{sc}",
                                name=f"vt{sc}")
                     for sc in range(SC)]

            # ---- QT/KT projection unit: one (pair, st, q|k) tile ----
            def emit_qkt_unit(p, st, which):
                ssl = slice(512 * st, 512 * (st + 1))
                wall, dst = (wqall, qt_sb[p]) if which == 0 else \
                    (wkall, kt_sb[p])
                ps = pspool.tile([128, 512], F32, tag="mm", name="qkps",
                                 bufs=2)
                for cc in range(CCH):
                    nc.tensor.matmul(
                        ps[:], w_c(wall, cc)[:, 128 * p:128 * (p + 1)],
                        xt_c(cc)[:, ssl],
                        start=(cc == 0), stop=(cc == CCH - 1))
                nc.vector.tensor_copy(dst[:, ssl], ps[:])

            # ---- pair-0 st0/st1 ramp: tile-minor over 4 accumulators,
            # cc ascending, chasing the half-A chunk-pair DMAs
            ramp = [(0, 0, "sps"), (0, 1, "sps"), (1, 0, "ot"), (1, 1, "ot")]
            pss = [pspool.tile([128, 512], F32, tag=ptag, name="rampps",
                               bufs=2) for _, _, ptag in ramp]
            for cc in range(CCH):
                for (st, which, _), ps in zip(ramp, pss):
                    wall = wqall if which == 0 else wkall
                    nc.tensor.matmul(
                        ps[:], w_c(wall, cc)[:, 0:128],
                        xt_c(cc)[:, 512 * st:512 * (st + 1)],
                        start=(cc == 0), stop=(cc == CCH - 1))
            for (st, which, _), ps in zip(ramp, pss):
                dst = qt_sb[0] if which == 0 else kt_sb[0]
                nc.vector.tensor_copy(dst[:, 512 * st:512 * (st + 1)], ps[:])

            # ---- V strip: V = x @ wv in [s, d] layout + ones column ----
            def emit_v_strip(sc):
                vt = vt_sb[sc]
                nc.gpsimd.memset(vt[:], 1.0)
                ps = pspool.tile([128, GW], F32, tag="mm", name="vps",
                                 bufs=2)
                for cc in range(CCH):
                    nc.tensor.matmul(
                        ps[:], xt_c(cc)[:, 128 * sc:128 * (sc + 1)],
                        w_c(wvall, cc)[:],
                        start=(cc == 0), stop=(cc == CCH - 1))
                vt_v = vt[:, :].rearrange("p (h d) -> p h d", h=HG)[:, :, 0:64]
                ps_v = ps[:, :].rearrange("p (h d) -> p h d", h=HG)
                nc.vector.tensor_copy(vt_v, ps_v)

            for sc in range(4):
                emit_v_strip(sc)

            # ---- out-proj chunk: out[s-chunk,:] = sum_p OTn_p.T @ wp_p ----
            def emit_out_chunk(sc):
                outst = wpool.tile([128, C], BF16, tag="outst", name="outst")
                for half in range(2):
                    pp = pspool.tile([128, 512], F32, tag="mm", name="pp",
                                     bufs=2)
                    for p in range(NPAIR):
                        nc.tensor.matmul(
                            pp[:], otn_sb[p][:, 128 * sc:128 * (sc + 1)],
                            wpall[:, C * p + 512 * half:
                                  C * p + 512 * (half + 1)],
                            start=(p == 0), stop=(p == NPAIR - 1))
                    nc.vector.tensor_copy(
                        outst[:, 512 * half:512 * (half + 1)], pp[:])
                nc.sync.dma_start(out[128 * sc:128 * (sc + 1), :], outst[:])

            # ---- per head-pair attention, k-block granular ----
            deferred_norm = [None]  # phase-2 closure from the previous j

            def run_deferred():
                if deferred_norm[0] is not None:
                    deferred_norm[0]()
                    deferred_norm[0] = None

            for p in range(NPAIR):
                qt, kt = qt_sb[p], kt_sb[p]
                for j in range(NQT):
                    nkb = 4 * (j + 1)  # causal: only k-blocks 0..nkb-1
                    # filler units for this (p, j) window
                    fillers = []
                    if p == 0 and j < 3:
                        for sc in range(4 * (j + 1), 4 * (j + 2)):
                            fillers.append(lambda sc=sc: emit_v_strip(sc))
                    if j < 2:
                        # own (st2, st3) QT/KT: 2 units per j
                        for which in range(2):
                            fillers.append(
                                lambda st=j + 2, w=which, pp_=p:
                                emit_qkt_unit(pp_, st, w))
                    elif p < 3:
                        # next pair's (st0, st1): 2 units per j
                        for which in range(2):
                            fillers.append(
                                lambda st=j - 2, w=which, pp_=p + 1:
                                emit_qkt_unit(pp_, st, w))
                    fill_at = {}
                    for fi in range(len(fillers)):
                        g = min(nkb - 1, (fi * nkb) // max(1, len(fillers)))
                        fill_at.setdefault(g, []).append(fillers[fi])
                    if p == 3 and j >= 1:
                        # out-proj chunks of q-tile j-1 in the last two
                        # slots (their otn inputs come from the previous
                        # normalize; placing them early would stall the
                        # in-order PE)
                        for i, sc in enumerate(range(4 * (j - 1), 4 * j)):
                            fill_at.setdefault(nkb - 2 + (i % 2), []).append(
                                lambda sc=sc: emit_out_chunk(sc))

                    ot = [pspool.tile([65, 512], F32, tag="ot", name="ot",
                                      bufs=2) for _ in range(2)]

                    def emit_av(kb, pt, j=j, nkb=nkb, ot=ot, p=p):
                        o = 128 * (kb - 4 * j) if kb >= 4 * j else 0
                        for h in range(2):
                            nc.tensor.matmul(
                                ot[h][:, o:512],
                                vt_sb[kb][:, 65 * (2 * p + h):
                                          65 * (2 * p + h) + 65],
                                pt[:, 512 * h + o:512 * (h + 1)],
                                start=(kb == 0), stop=(kb == nkb - 1))

                    pending = []
                    for kb in range(nkb):
                        diag = kb >= 4 * j
                        o = 128 * (kb - 4 * j) if diag else 0
                        # scores (transposed), both heads packed side by
                        # side in one psum tile: h at cols [512h+o:512h+512]
                        sp = pspool.tile([128, 1024], F32, tag="sps",
                                         name="sps", bufs=2)
                        for h in range(2):
                            hsl = slice(64 * h, 64 * (h + 1))
                            nc.tensor.matmul(
                                sp[:, 512 * h + o:512 * (h + 1)],
                                kt[hsl, 128 * kb:128 * (kb + 1)],
                                qt[hsl, 512 * j + o:512 * (j + 1)],
                                start=True, stop=True)
                        pt = ptpool.tile([128, 1024], BF16, tag="pt",
                                         name="pt")
                        if o > 0:
                            # one ACTIVATE over both heads' live columns
                            spv = sp[:, :].rearrange(
                                "p (h f) -> p h f", h=2)[:, :, o:512]
                            ptv = pt[:, :].rearrange(
                                "p (h f) -> p h f", h=2)[:, :, o:512]
                            nc.scalar.activation(ptv, spv, EXP, scale=SCALE)
                        else:
                            nc.scalar.activation(pt[:], sp[:], EXP,
                                                 scale=SCALE)
                        if diag:
                            for h in range(2):
                                csl = slice(512 * h + o, 512 * (h + 1))
                                nc.vector.tensor_mul(
                                    pt[:, csl], pt[:, csl],
                                    mask_sb[:, 0:512 - o])
                        pending.append((kb, pt))
                        if len(pending) > 4:
                            emit_av(*pending.pop(0))
                        if kb == 1:
                            run_deferred()
                        for f in fill_at.get(kb, []):
                            f()
                    for item in pending:
                        emit_av(*item)

                    # ---- normalize phase 1: numerators to SBUF (releases
                    # psum), denominators to partition 0 via tiny
                    # vector-issued DMAs, merged reciprocal, broadcasts.
                    qsl = slice(512 * j, 512 * (j + 1))
                    s64a = wpool.tile([65, 512], F32, tag="s64a", name="s64a")
                    s64b = wpool.tile([65, 512], F32, tag="s64b", name="s64b")
                    nc.vector.tensor_copy(s64a[:, :], ot[0][:, :])
                    nc.vector.tensor_copy(s64b[:, :], ot[1][:, :])
                    dens = wpool.tile([1, 1024], F32, tag="dens", name="dens")
                    nc.gpsimd.dma_start(dens[0:1, 0:512], s64a[64:65, :])
                    nc.gpsimd.dma_start(dens[0:1, 512:1024], s64b[64:65, :])
                    inv = wpool.tile([1, 1024], F32, tag="inv", name="inv")
                    nc.vector.reciprocal_approx_fast(inv[0:1, :],
                                                     dens[0:1, :])
                    bcs0 = wpool.tile([64, 512], F32, tag="bcs0", name="bcs0")
                    bcs1 = wpool.tile([64, 512], F32, tag="bcs1", name="bcs1")
                    nc.gpsimd.partition_broadcast(bcs0[:], inv[0:1, 0:512])
                    nc.gpsimd.partition_broadcast(bcs1[:], inv[0:1, 512:1024])

                    # ---- phase 2 (deferred one q-tile): multiplies on DVE
                    # (broadcast long done by then), partition-shifting oth
                    # copy issued from the gpsimd queue.
                    def phase2(p=p, qsl=qsl, s64a=s64a, s64b=s64b,
                               bcs0=bcs0, bcs1=bcs1):
                        nc.vector.tensor_mul(otn_sb[p][0:64, qsl],
                                             s64a[0:64, :], bcs0[:])
                        oth = wpool.tile([64, 512], BF16, tag="oth",
                                         name="oth")
                        nc.vector.tensor_mul(oth[:], s64b[0:64, :], bcs1[:])
                        nc.gpsimd.dma_start(otn_sb[p][64:128, qsl], oth[:])

                    deferred_norm[0] = phase2

            # ---- tail: chunks 12-14 pre-accumulate pairs 0-2 while the
            # last normalize chain (otn[3] q-tile 3) is still in flight,
            # then finish with pair 3's contribution; chunk 15 runs whole.
            tail_pps = {}
            for sc, ptag in ((12, "mm"), (13, "sps"), (14, "ot")):
                pps = []
                for half in range(2):
                    pp = pspool.tile([128, 512], F32, tag=ptag, name="pp",
                                     bufs=2)
                    for p in range(3):
                        nc.tensor.matmul(
                            pp[:], otn_sb[p][:, 128 * sc:128 * (sc + 1)],
                            wpall[:, C * p + 512 * half:
                                  C * p + 512 * (half + 1)],
                            start=(p == 0), stop=False)
                    pps.append(pp)
                tail_pps[sc] = pps
            run_deferred()
            for sc in (12, 13, 14):
                outst = wpool.tile([128, C], BF16, tag="outst", name="outst")
                for half, pp in enumerate(tail_pps[sc]):
                    nc.tensor.matmul(
                        pp[:], otn_sb[3][:, 128 * sc:128 * (sc + 1)],
                        wpall[:, C * 3 + 512 * half:C * 3 + 512 * (half + 1)],
                        start=False, stop=True)
                    nc.vector.tensor_copy(
                        outst[:, 512 * half:512 * (half + 1)], pp[:])
                nc.sync.dma_start(out[128 * sc:128 * (sc + 1), :], outst[:])
            emit_out_chunk(15)

    nc.compile()
    return nc


_NC_CACHE = None


def _get_nc():
    global _NC_CACHE
    if _NC_CACHE is None:
        _NC_CACHE = build_nc()
    return _NC_CACHE


def make_in_maps(x, w_qkv, w_proj):
    """Shard full inputs into the 8 per-core input dicts."""
    bf = ml_dtypes.bfloat16
    mask01 = (np.arange(128)[:, None] <= np.arange(512)[None, :]) \
        .astype(bf)
    in_maps = []
    for core in range(N_CORES):
        b, g = core // 2, core % 2
        gsl = slice(GW * g, GW * (g + 1))
        in_maps.append({
            "xT": np.ascontiguousarray(x[b].T).astype(bf),
            "wq": np.ascontiguousarray(w_qkv[:, 0 * C:1 * C][:, gsl]).astype(bf),
            "wk": np.ascontiguousarray(w_qkv[:, 1 * C:2 * C][:, gsl]).astype(bf),
            "wv": np.ascontiguousarray(w_qkv[:, 2 * C:3 * C][:, gsl]).astype(bf),
            "wp": np.ascontiguousarray(w_proj[gsl, :]).astype(bf),
            "mask": mask01,
        })
    return in_maps


def kernel(x, w_qkv, w_proj, b_proj, _profile=False):
    import os
    if not _profile:
        # the NTFF trace path needs modules absent from this image;
        # make sure an inherited BASS_TRACE can't route us into it
        os.environ["BASS_NEVER_TRACE"] = "1"
    else:
        os.environ.pop("BASS_NEVER_TRACE", None)
    x = np.asarray(x, np.float32)
    w_qkv = np.asarray(w_qkv, np.float32)
    w_proj = np.asarray(w_proj, np.float32)
    b_proj = np.asarray(b_proj, np.float32)

    nc = _get_nc()
    in_maps = make_in_maps(x, w_qkv, w_proj)
    res = run_bass_kernel_spmd(nc, in_maps, core_ids=list(range(N_CORES)),
                               trace=_profile)
    partials = [np.asarray(res.results[c]["out"], np.float32)
                for c in range(N_CORES)]
    out = np.empty((B, S, C), np.float32)
    for b in range(B):
        out[b] = partials[2 * b] + partials[2 * b + 1] + b_proj
    if _profile:
        return out, res
    return out


# revision 16
# speedup vs baseline: 1.3412x; 1.0146x over previous
"""Causal multi-head flash-attention block (QKV proj + attention + out proj)
for Trainium2, distributed over 8 NeuronCores.

Sharding: data-parallel over batch (B=4) x tensor-parallel over head groups
(16 heads -> 2 groups of 8). Core c handles batch c//2, head group c%2.
Each core computes a partial output projection (its 8 heads' contribution);
the host sums the two partials per batch and adds the bias.

v3 schedule notes: the attention inner loop is ACT(exp)-bound, and the PE
executes its queue strictly in order, so independent matmul work is
interleaved INTO the attention k-block loop to keep the PE dense:
  - each pair's (st2,st3) QT/KT units fill its own j0/j1, the NEXT pair's
    (st0,st1) units fill j2/j3 (so casts land well before the pair switch),
  - V strips fill pair 0, out-proj chunks fill pair 3's late slots,
  - inputs arrive via 13 grouped multi-chunk DMAs (3D access patterns) so
    the sync queue isn't issue-bound; pair-0 st0/st1 QT/KT runs tile-minor
    over the arriving chunk-pairs (PE dense from ~11us).
Scores/exp are k-block granular: one ACTIVATE per k-block covering both
packed heads, restricted to causally live columns on diagonal blocks.
The softmax-normalize chain is split in two phases and issues its DMAs
from the producer engines' own queues (dens from vector, the oth
partition-shift from gpsimd) so no FIFO head-of-line-blocks another
engine's PE-feeding work; the multiplies run late (deferred one q-tile)
on the DVE after the gpsimd broadcast is long done.

Per-core kernel (all matmuls bf16 operands, fp32 PSUM accumulate):
  - QKV proj from host-pretransposed x^T: Q^T,K^T in [d, s] layout, V in
    [s, d] layout with a ones-column per head (rowsum trick).
  - Scores transposed: ST[k,q] via lhsT=KT-block, rhs=QT; two heads packed
    via PE row tiling (K=64 each, partitions 0:64 / 64:128, one XBUS).
  - softmax without max-subtraction (logits ~ N(0,1)); exp on ACT with the
    1/8 scale folded in; causal 0/1 mask multiply post-exp on diagonal
    blocks; fully-masked blocks skipped.
  - AV: lhsT = V-tile [128, 65] (65th col = ones -> row 64 of PSUM is the
    softmax denominator), rhs = P^T tiles.
  - Normalize: psum row 64 -> partition 0 via tiny DMAs, merged
    reciprocal_approx_fast, gpsimd partition_broadcast, DVE multiplies.
  - Output proj from O^T [head*64+d, s] chunks against w_proj rows;
    partial outputs written bf16 (host sums in f32).
"""

import numpy as np
import ml_dtypes

import concourse.bass as bass
import concourse.bacc as bacc
import concourse.mybir as mybir
import concourse.tile as tile
from concourse.bass_utils import run_bass_kernel_spmd

F32 = mybir.dt.float32
BF16 = mybir.dt.bfloat16
EXP = mybir.ActivationFunctionType.Exp

# Problem constants (hardcoded per contract)
B, S, C = 4, 2048, 1024
NH, D = 16, 64
SCALE = D ** -0.5
N_CORES = 8
HG = NH // 2          # heads per core (head group)
NPAIR = HG // 2       # head pairs per core
CCH = C // 128        # contraction chunks for QKV proj
SC = S // 128         # s-chunks (also k-blocks count)
NQT = S // 512        # q-tiles of 512
GW = C // 2           # group width of qkv output (8 heads * 64)


def build_nc():
    nc = bacc.Bacc("TRN2", target_bir_lowering=False, debug=False)

    xT = nc.dram_tensor("xT", [C, S], BF16, kind="ExternalInput")
    wq = nc.dram_tensor("wq", [C, GW], BF16, kind="ExternalInput")
    wk = nc.dram_tensor("wk", [C, GW], BF16, kind="ExternalInput")
    wv = nc.dram_tensor("wv", [C, GW], BF16, kind="ExternalInput")
    wp = nc.dram_tensor("wp", [GW, C], BF16, kind="ExternalInput")
    mask = nc.dram_tensor("mask", [128, 512], BF16, kind="ExternalInput")
    out = nc.dram_tensor("out", [S, C], BF16, kind="ExternalOutput")

    with tile.TileContext(nc) as tc:
        with (
            tc.tile_pool(name="const", bufs=1) as cpool,
            tc.tile_pool(name="pt", bufs=8) as ptpool,
            tc.tile_pool(name="work", bufs=2) as wpool,
            tc.tile_pool(name="ps", bufs=2, space="PSUM") as pspool,
        ):
            # ---- persistent tiles; grouped input DMAs (3D APs) ----
            # chunk cc of a weight lives at cols [512cc:512(cc+1)];
            # chunk cc of xT at cols [2048cc:2048(cc+1)].
            wqall = cpool.tile([128, GW * CCH], BF16, tag="wqall", name="wqall")
            wkall = cpool.tile([128, GW * CCH], BF16, tag="wkall", name="wkall")
            wvall = cpool.tile([128, GW * CCH], BF16, tag="wvall", name="wvall")
            xtall = cpool.tile([128, S * CCH], BF16, tag="xtall", name="xtall")
            mask_sb = cpool.tile([128, 512], BF16, tag="mask", name="maskt")

            # per-chunk DMAs: dram reads stay sequential (a grouped 3D AP
            # with partition-outer ordering turns into 2KB strided bursts
            # at ~51 GB/s -- measured). Arrival order: wq/wk/mask, xT
            # half-A (the st0/st1 ramp chases these), wv, xT half-B, wp.
            # issue from three engine queues in parallel (the ~0.65us
            # per-issue cost on one queue would gate the ramp): wq from
            # scalar, wk from gpsimd, xT/wv/wp/mask from sync
            for cc in range(CCH):
                nc.scalar.dma_start(wqall[:, GW * cc:GW * (cc + 1)],
                                    wq[128 * cc:128 * (cc + 1), :])
            for cc in range(CCH):
                nc.gpsimd.dma_start(wkall[:, GW * cc:GW * (cc + 1)],
                                    wk[128 * cc:128 * (cc + 1), :])
            nc.sync.dma_start(mask_sb[:], mask[:, :])
            for cc in range(CCH):
                nc.sync.dma_start(xtall[:, S * cc:S * cc + 1024],
                                  xT[128 * cc:128 * (cc + 1), 0:1024])
            for cc in range(CCH):
                nc.sync.dma_start(wvall[:, GW * cc:GW * (cc + 1)],
                                  wv[128 * cc:128 * (cc + 1), :])
            for cc in range(CCH):
                nc.sync.dma_start(xtall[:, S * cc + 1024:S * (cc + 1)],
                                  xT[128 * cc:128 * (cc + 1), 1024:2048])
            wpall = cpool.tile([128, C * NPAIR], BF16, tag="wpall", name="wpall")
            for p in range(NPAIR):
                nc.sync.dma_start(wpall[:, C * p:C * (p + 1)],
                                  wp[128 * p:128 * (p + 1), :])

            def xt_c(cc):      # xT chunk cc, [128, S]
                return xtall[:, S * cc:S * (cc + 1)]

            def w_c(wall, cc):  # weight chunk cc, [128, GW]
                return wall[:, GW * cc:GW * (cc + 1)]

            # preload the ACT exp table set while input DMAs run
            actwarm = cpool.tile([1, 8], F32, tag="actwarm", name="actwarm")
            nc.vector.memset(actwarm[:], 0.0)
            nc.scalar.activation(actwarm[:], actwarm[:], EXP)

            qt_sb = [cpool.tile([128, S], BF16, tag=f"qt{p}", name=f"qt{p}")
                     for p in range(NPAIR)]
            kt_sb = [cpool.tile([128, S], BF16, tag=f"kt{p}", name=f"kt{p}")
                     for p in range(NPAIR)]
            otn_sb = [cpool.tile([128, S], BF16, tag=f"otn{p}", name=f"otn{p}")
                      for p in range(NPAIR)]
            vt_sb = [cpool.tile([128, 65 * HG], BF16, tag=f"vt{sc}",
                                name=f"vt{sc}")
                     for sc in range(SC)]

            # ---- QT/KT projection unit: one (pair, st, q|k) tile ----
            def emit_qkt_unit(p, st, which):
                ssl = slice(512 * st, 512 * (st + 1))
                wall, dst = (wqall, qt_sb[p]) if which == 0 else \
                    (wkall, kt_sb[p])
                ps = pspool.tile([128, 512], F32, tag="mm", name="qkps",
                                 bufs=2)
                for cc in range(CCH):
                    nc.tensor.matmul(
                        ps[:], w_c(wall, cc)[:, 128 * p:128 * (p + 1)],
                        xt_c(cc)[:, ssl],
                        start=(cc == 0), stop=(cc == CCH - 1))
                nc.vector.tensor_copy(dst[:, ssl], ps[:])

            # ---- pair-0 st0/st1 ramp: tile-minor over 4 accumulators,
            # cc ascending, chasing the half-A chunk-pair DMAs
            ramp = [(0, 0, "sps"), (0, 1, "sps"), (1, 0, "ot"), (1, 1, "ot")]
            pss = [pspool.tile([128, 512], F32, tag=ptag, name="rampps",
                               bufs=2) for _, _, ptag in ramp]
            for cc in range(CCH):
                for (st, which, _), ps in zip(ramp, pss):
                    wall = wqall if which == 0 else wkall
                    nc.tensor.matmul(
                        ps[:], w_c(wall, cc)[:, 0:128],
                        xt_c(cc)[:, 512 * st:512 * (st + 1)],
                        start=(cc == 0), stop=(cc == CCH - 1))
            for (st, which, _), ps in zip(ramp, pss):
                dst = qt_sb[0] if which == 0 else kt_sb[0]
                nc.vector.tensor_copy(dst[:, 512 * st:512 * (st + 1)], ps[:])

            # ---- V strip: V = x @ wv in [s, d] layout + ones column ----
            def emit_v_strip(sc):
                vt = vt_sb[sc]
                nc.gpsimd.memset(vt[:], 1.0)
                ps = pspool.tile([128, GW], F32, tag="mm", name="vps",
                                 bufs=2)
                for cc in range(CCH):
                    nc.tensor.matmul(
                        ps[:], xt_c(cc)[:, 128 * sc:128 * (sc + 1)],
                        w_c(wvall, cc)[:],
                        start=(cc == 0), stop=(cc == CCH - 1))
                vt_v = vt[:, :].rearrange("p (h d) -> p h d", h=HG)[:, :, 0:64]
                ps_v = ps[:, :].rearrange("p (h d) -> p h d", h=HG)
                nc.vector.tensor_copy(vt_v, ps_v)

            for sc in range(4):
                emit_v_strip(sc)

            # ---- out-proj chunk: out[s-chunk,:] = sum_p OTn_p.T @ wp_p ----
            def emit_out_chunk(sc):
                outst = wpool.tile([128, C], BF16, tag="outst", name="outst")
                for half in range(2):
                    pp = pspool.tile([128, 512], F32, tag="mm", name="pp",
                                     bufs=2)
                    for p in range(NPAIR):
                        nc.tensor.matmul(
                            pp[:], otn_sb[p][:, 128 * sc:128 * (sc + 1)],
                            wpall[:, C * p + 512 * half:
                                  C * p + 512 * (half + 1)],
                            start=(p == 0), stop=(p == NPAIR - 1))
                    nc.vector.tensor_copy(
                        outst[:, 512 * half:512 * (half + 1)], pp[:])
                nc.sync.dma_start(out[128 * sc:128 * (sc + 1), :], outst[:])

            # ---- per head-pair attention, k-block granular ----
            deferred_norm = [None]  # phase-2 closure from the previous j

            def run_deferred():
                if deferred_norm[0] is not None:
                    deferred_norm[0]()
                    deferred_norm[0] = None

            for p in range(NPAIR):
                qt, kt = qt_sb[p], kt_sb[p]
                for j in range(NQT):
                    nkb = 4 * (j + 1)  # causal: only k-blocks 0..nkb-1
                    # filler units for this (p, j) window
                    fillers = []
                    if p == 0 and j < 3:
                        for sc in range(4 * (j + 1), 4 * (j + 2)):
                            fillers.append(lambda sc=sc: emit_v_strip(sc))
                    if j < 2:
                        # own (st2, st3) QT/KT: 2 units per j
                        for which in range(2):
                            fillers.append(
                                lambda st=j + 2, w=which, pp_=p:
                                emit_qkt_unit(pp_, st, w))
                    elif p < 3:
                        # next pair's (st0, st1): 2 units per j
                        for which in range(2):
                            fillers.append(
                                lambda st=j - 2, w=which, pp_=p + 1:
                                emit_qkt_unit(pp_, st, w))
                    fill_at = {}
                    for fi in range(len(fillers)):
                        g = min(nkb - 1, (fi * nkb) // max(1, len(fillers)))
                        fill_at.setdefault(g, []).append(fillers[fi])
                    if p == 3 and j >= 1:
                        # out-proj chunks of q-tile j-1 in the last two
                        # slots (their otn inputs come from the previous
                        # normalize; placing them early would stall the
                        # in-order PE)
                        for i, sc in enumerate(range(4 * (j - 1), 4 * j)):
                            fill_at.setdefault(nkb - 2 + (i % 2), []).append(
                                lambda sc=sc: emit_out_chunk(sc))

                    ot = [pspool.tile([65, 512], F32, tag="ot", name="ot",
                                      bufs=2) for _ in range(2)]

                    def emit_av(kb, pt, j=j, nkb=nkb, ot=ot, p=p):
                        o = 128 * (kb - 4 * j) if kb >= 4 * j else 0
                        for h in range(2):
                            nc.tensor.matmul(
                                ot[h][:, o:512],
                                vt_sb[kb][:, 65 * (2 * p + h):
                                          65 * (2 * p + h) + 65],
                                pt[:, 512 * h + o:512 * (h + 1)],
                                start=(kb == 0), stop=(kb == nkb - 1))

                    pending = []
                    for kb in range(nkb):
                        diag = kb >= 4 * j
                        o = 128 * (kb - 4 * j) if diag else 0
                        # scores (transposed), both heads packed side by
                        # side in one psum tile: h at cols [512h+o:512h+512]
                        sp = pspool.tile([128, 1024], F32, tag="sps",
                                         name="sps", bufs=2)
                        for h in range(2):
                            hsl = slice(64 * h, 64 * (h + 1))
                            nc.tensor.matmul(
                                sp[:, 512 * h + o:512 * (h + 1)],
                                kt[hsl, 128 * kb:128 * (kb + 1)],
                                qt[hsl, 512 * j + o:512 * (j + 1)],
                                start=True, stop=True)
                        pt = ptpool.tile([128, 1024], BF16, tag="pt",
                                         name="pt")
                        if o > 0:
                            # one ACTIVATE over both heads' live columns
                            spv = sp[:, :].rearrange(
                                "p (h f) -> p h f", h=2)[:, :, o:512]
                            ptv = pt[:, :].rearrange(
                                "p (h f) -> p h f", h=2)[:, :, o:512]
                            nc.scalar.activation(ptv, spv, EXP, scale=SCALE)
                        else:
                            nc.scalar.activation(pt[:], sp[:], EXP,
                                                 scale=SCALE)
                        if diag:
                            for h in range(2):
                                csl = slice(512 * h + o, 512 * (h + 1))
                                nc.vector.tensor_mul(
                                    pt[:, csl], pt[:, csl],
                                    mask_sb[:, 0:512 - o])
                        pending.append((kb, pt))
                        if len(pending) > 4:
                            emit_av(*pending.pop(0))
                        if kb == (5 if nkb > 5 else nkb - 1):
                            run_deferred()
                        for f in fill_at.get(kb, []):
                            f()
                    for item in pending:
                        emit_av(*item)

                    # ---- normalize phase 1: denominators (psum row 64)
                    # straight to partition 0 via tiny gpsimd-issued DMAs,
                    # bf16 numerator casts release the psum banks, merged
                    # reciprocal, one merged bf16 broadcast.
                    qsl = slice(512 * j, 512 * (j + 1))
                    s64a = wpool.tile([65, 512], F32, tag="s64a", name="s64a")
                    s64b = wpool.tile([65, 512], F32, tag="s64b", name="s64b")
                    nc.vector.tensor_copy(s64a[:, :], ot[0][:, :])
                    nc.vector.tensor_copy(s64b[:, :], ot[1][:, :])
                    dens = wpool.tile([1, 1024], F32, tag="dens", name="dens")
                    nc.gpsimd.dma_start(dens[0:1, 0:512], s64a[64:65, :])
                    nc.gpsimd.dma_start(dens[0:1, 512:1024], s64b[64:65, :])
                    inv = wpool.tile([1, 1024], F32, tag="inv", name="inv")
                    nc.vector.reciprocal_approx_fast(inv[0:1, :],
                                                     dens[0:1, :])
                    bcs = wpool.tile([64, 1024], F32, tag="bcs", name="bcs")
                    nc.gpsimd.partition_broadcast(bcs[:], inv[0:1, :])

                    # ---- phase 2 (deferred into the next q-tile, after
                    # the broadcast has finished): multiplies on DVE,
                    # partition-shifting oth copy from the gpsimd queue.
                    def phase2(p=p, qsl=qsl, s64a=s64a, s64b=s64b, bcs=bcs):
                        nc.vector.tensor_mul(otn_sb[p][0:64, qsl],
                                             s64a[0:64, :], bcs[:, 0:512])
                        oth = wpool.tile([64, 512], BF16, tag="oth",
                                         name="oth")
                        nc.vector.tensor_mul(oth[:], s64b[0:64, :],
                                             bcs[:, 512:1024])
                        nc.gpsimd.dma_start(otn_sb[p][64:128, qsl], oth[:])

                    deferred_norm[0] = phase2

            # ---- tail: chunks 12-14 pre-accumulate pairs 0-2 while the
            # last normalize chain (otn[3] q-tile 3) is still in flight,
            # then finish with pair 3's contribution; chunk 15 runs whole.
            tail_pps = {}
            for sc, ptag in ((12, "mm"), (13, "sps"), (14, "ot"), (15, "mm")):
                pps = []
                for half in range(2):
                    pp = pspool.tile([128, 512], F32, tag=ptag, name="pp",
                                     bufs=2)
                    for p in range(3):
                        nc.tensor.matmul(
                            pp[:], otn_sb[p][:, 128 * sc:128 * (sc + 1)],
                            wpall[:, C * p + 512 * half:
                                  C * p + 512 * (half + 1)],
                            start=(p == 0), stop=False)
                    pps.append(pp)
                tail_pps[sc] = pps
            run_deferred()
            for sc in (12, 13, 14, 15):
                outst = wpool.tile([128, C], BF16, tag="outst", name="outst")
                for half, pp in enumerate(tail_pps[sc]):
                    nc.tensor.matmul(
                        pp[:], otn_sb[3][:, 128 * sc:128 * (sc + 1)],
                        wpall[:, C * 3 + 512 * half:C * 3 + 512 * (half + 1)],
                        start=False, stop=True)
                    nc.vector.tensor_copy(
                        outst[:, 512 * half:512 * (half + 1)], pp[:])
                nc.sync.dma_start(out[128 * sc:128 * (sc + 1), :], outst[:])

    nc.compile()
    return nc


_NC_CACHE = None


def _get_nc():
    global _NC_CACHE
    if _NC_CACHE is None:
        _NC_CACHE = build_nc()
    return _NC_CACHE


def make_in_maps(x, w_qkv, w_proj):
    """Shard full inputs into the 8 per-core input dicts."""
    bf = ml_dtypes.bfloat16
    mask01 = (np.arange(128)[:, None] <= np.arange(512)[None, :]) \
        .astype(bf)
    in_maps = []
    for core in range(N_CORES):
        b, g = core // 2, core % 2
        gsl = slice(GW * g, GW * (g + 1))
        in_maps.append({
            "xT": np.ascontiguousarray(x[b].T).astype(bf),
            "wq": np.ascontiguousarray(w_qkv[:, 0 * C:1 * C][:, gsl]).astype(bf),
            "wk": np.ascontiguousarray(w_qkv[:, 1 * C:2 * C][:, gsl]).astype(bf),
            "wv": np.ascontiguousarray(w_qkv[:, 2 * C:3 * C][:, gsl]).astype(bf),
            "wp": np.ascontiguousarray(w_proj[gsl, :]).astype(bf),
            "mask": mask01,
        })
    return in_maps


def kernel(x, w_qkv, w_proj, b_proj, _profile=False):
    import os
    if not _profile:
        # the NTFF trace path needs modules absent from this image;
        # make sure an inherited BASS_TRACE can't route us into it
        os.environ["BASS_NEVER_TRACE"] = "1"
    else:
        os.environ.pop("BASS_NEVER_TRACE", None)
    x = np.asarray(x, np.float32)
    w_qkv = np.asarray(w_qkv, np.float32)
    w_proj = np.asarray(w_proj, np.float32)
    b_proj = np.asarray(b_proj, np.float32)

    nc = _get_nc()
    in_maps = make_in_maps(x, w_qkv, w_proj)
    res = run_bass_kernel_spmd(nc, in_maps, core_ids=list(range(N_CORES)),
                               trace=_profile)
    partials = [np.asarray(res.results[c]["out"], np.float32)
                for c in range(N_CORES)]
    out = np.empty((B, S, C), np.float32)
    for b in range(B):
        out[b] = partials[2 * b] + partials[2 * b + 1] + b_proj
    if _profile:
        return out, res
    return out
